# revision 11
# baseline (speedup 1.0000x reference)
"""Multi-head attention (B=2, N=2048, D=1024, H=16) on 8 Trainium2 cores.

Sharding: data-parallel over batch (cores 0-3 -> b=0, cores 4-7 -> b=1) and
tensor-parallel over heads (4 heads per core = 256 of 1024 QKV/O channels).
Each core computes its 4 heads' attention plus a partial output projection;
the host sums the 4 partials per batch and adds bo.

v3a pipeline (per core): baseline attention structure + need-ordered
chunked input DMA on the 3 rings so the first K-proj matmul fires ~9us in,
and ScalarE keeps its ring free during attention (stores ride sync/gpsimd).
"""

import numpy as np

import concourse.bass as bass
import concourse.bacc as bacc
import concourse.tile as tile
from concourse import mybir
from concourse.bass_utils import run_bass_kernel_spmd

F32 = mybir.dt.float32
BF16 = mybir.dt.bfloat16
AF = mybir.ActivationFunctionType

B, N, D, H, HD = 2, 2048, 1024, 16, 64
E = 256            # channels per core (4 heads * 64)
DC = D // 128      # 8 contraction chunks for projections
NB = N // 128      # 16 token blocks / k chunks
G = NB // 2        # 8 two-k-block groups per attention block
QC = 512           # query chunk
NQC = N // QC      # 4 query chunks
SCALE = 1.0 / np.sqrt(HD)
DT = BF16


def _emit(nc):
    # all big inputs arrive host-swizzled: per-partition contiguous rows so
    # the DMA descriptors are large (partition p owns columns p of every
    # 128-row chunk)
    xT = nc.dram_tensor("xT", [128, DC * N], DT, kind="ExternalInput")
    wqT = nc.dram_tensor("wqT", [128, DC * E], DT, kind="ExternalInput")
    wkT = nc.dram_tensor("wkT", [128, DC * E], DT, kind="ExternalInput")
    wvT = nc.dram_tensor("wvT", [128, DC * E], DT, kind="ExternalInput")
    woT = nc.dram_tensor("woT", [128, 2 * D], DT, kind="ExternalInput")
    bq2 = nc.dram_tensor("bq2", [128, 2], F32, kind="ExternalInput")
    bk2 = nc.dram_tensor("bk2", [128, 2], F32, kind="ExternalInput")
    bv1 = nc.dram_tensor("bv1", [E], F32, kind="ExternalInput")
    vones = nc.dram_tensor("vones", [128, NB, 4], DT, kind="ExternalInput")
    out = nc.dram_tensor("out", [N, D], DT, kind="ExternalOutput")

    with tile.TileContext(nc) as tc:
        with tc.tile_pool(name="per", bufs=1) as per, \
             tc.tile_pool(name="wp", bufs=6) as wp, \
             tc.tile_pool(name="dn", bufs=2) as dn, \
             tc.tile_pool(name="up", bufs=2) as up, \
             tc.tile_pool(name="op", bufs=6) as op:

            # ---- persistent SBUF tiles ----
            # x arrives as per-d-chunk tiles (chunk 0 split in half) so each
            # chunk's projection matmuls fire as soon as its DMA lands;
            # weights arrive as 4-chunk halves.  Separate tiles per DMA so
            # no false write-order deps serialize them.
            xt0 = [per.tile([128, N // 2], DT, name=f"xt0{i}") for i in range(2)]
            xts = [per.tile([128, N], DT, name=f"xt{i}") for i in range(1, DC)]

            def xap(dc, lo, hi):
                if dc == 0:
                    half = N // 2
                    if hi <= half:
                        return xt0[0][:, lo:hi]
                    if lo >= half:
                        return xt0[1][:, lo - half:hi - half]
                    raise AssertionError("x chunk-0 slice crosses halves")
                return xts[dc - 1][:, lo:hi]

            wq2 = [per.tile([128, 4, E], DT, name=f"wq2{i}") for i in range(2)]
            wk2 = [per.tile([128, 4, E], DT, name=f"wk2{i}") for i in range(2)]
            wv2 = [per.tile([128, 4, E], DT, name=f"wv2{i}") for i in range(2)]
            wq = [wq2[dc // 4][:, dc % 4, :] for dc in range(DC)]
            wk = [wk2[dc // 4][:, dc % 4, :] for dc in range(DC)]
            wv = [wv2[dc // 4][:, dc % 4, :] for dc in range(DC)]
            wo = per.tile([128, 2, D], DT)            # WoT (e-chunk)
            qt = per.tile([128, 2, N], DT)            # Q^T: (pair, tokens)
            kt = per.tile([128, 2, N], DT)
            vp = per.tile([128, NB, 4, 128], DT)      # V natural + ones col
            at = per.tile([128, 2, N], DT)            # attn^T normalized
            bqs = per.tile([128, 2], F32)
            bks = per.tile([128, 2], F32)
            bvb = per.tile([128, E], F32)

            qs = [nc.sync, nc.gpsimd]
            # ---- input DMA: need-ordered chunked transfers on 3 rings ----
            # Only sync/scalar/gpsimd can drive DMA.  Each ring's first
            # transfers are the ones that gate the pre-phase: wk chunk-half
            # + x chunk 0 halves land first so the first K-proj matmul
            # fires ~9us in; x chunks are interleaved so they arrive
            # roughly in d-chunk order.
            def wslice(dram, i):
                return dram[:, 4 * i * E:4 * (i + 1) * E].rearrange(
                    "p (c e) -> p c e", e=E)

            bv_ap = bv1[:]
            # sync ring
            nc.sync.dma_start(out=wk2[0], in_=wslice(wkT, 0))
            nc.sync.dma_start(out=xt0[0], in_=xT[:, 0:N // 2])
            nc.sync.dma_start(out=wk2[1], in_=wslice(wkT, 1))
            nc.sync.dma_start(out=bqs, in_=bq2[:, :])
            nc.sync.dma_start(out=bks, in_=bk2[:, :])
            nc.sync.dma_start(out=xts[1], in_=xT[:, 2 * N:3 * N])
            nc.sync.dma_start(out=xts[4], in_=xT[:, 5 * N:6 * N])
            # scalar ring (free until the first exp ~25us in)
            nc.scalar.dma_start(out=wq2[0], in_=wslice(wqT, 0))
            nc.scalar.dma_start(out=xt0[1], in_=xT[:, N // 2:N])
            nc.scalar.dma_start(out=xts[2], in_=xT[:, 3 * N:4 * N])
            nc.scalar.dma_start(out=xts[5], in_=xT[:, 6 * N:7 * N])
            nc.scalar.dma_start(out=wq2[1], in_=wslice(wqT, 1))
            nc.scalar.dma_start(out=vp[:, :, :, HD:HD + 1],
                                in_=vones[:, :, :].rearrange(
                                    "p a (b o) -> p a b o", o=1))
            nc.scalar.dma_start(out=wo, in_=woT.rearrange(
                "p (c e) -> p c e", e=D))
            # gpsimd ring
            nc.gpsimd.dma_start(out=wv2[0], in_=wslice(wvT, 0))
            nc.gpsimd.dma_start(out=wv2[1], in_=wslice(wvT, 1))
            nc.gpsimd.dma_start(
                out=bvb,
                in_=bass.AP(tensor=bv_ap.tensor, offset=0, ap=[[0, 128], [1, E]]),
            )
            nc.gpsimd.dma_start(out=xts[0], in_=xT[:, N:2 * N])
            nc.gpsimd.dma_start(out=xts[3], in_=xT[:, 4 * N:5 * N])
            nc.gpsimd.dma_start(out=xts[6], in_=xT[:, 7 * N:8 * N])

            # ---- pre-phase: warmup + chunk-major first projections ----
            # K0 g0-3, Q0 g0, V nb0-1 accumulate concurrently in a dedicated
            # PSUM pool (7 banks); each d-chunk's matmuls fire as the chunk
            # arrives from HBM.
            with tc.tile_pool(name="pre", bufs=1, space="PSUM") as pre:
                pk = [pre.tile([128, 512], F32, tag=f"p{g}", name=f"pk{g}")
                      for g in range(4)]
                pq0 = pre.tile([128, 512], F32, tag="p4", name="pq0")
                pv01 = [pre.tile([128, E], F32, tag=f"p{5 + i}", name=f"pv{i}")
                        for i in range(2)]
                for dc in range(DC):
                    for g in range(4):
                        nc.tensor.matmul(
                            pk[g], wk[dc][:, 0:128],
                            xap(dc, g * 512, (g + 1) * 512),
                            start=(dc == 0), stop=(dc == DC - 1))
                    nc.tensor.matmul(
                        pq0, wq[dc][:, 0:128], xap(dc, 0, 512),
                        start=(dc == 0), stop=(dc == DC - 1))
                    for i in range(2):
                        nc.tensor.matmul(
                            pv01[i], xap(dc, i * 128, (i + 1) * 128),
                            wv[dc],
                            start=(dc == 0), stop=(dc == DC - 1))
                with tc.high_priority(offset=1 << 19):
                    for g in range(4):
                        nc.vector.tensor_scalar_add(
                            kt[:, 0, g * 512:(g + 1) * 512], pk[g],
                            bks[:, 0:1])
                    nc.vector.tensor_scalar_add(qt[:, 0, 0:512], pq0,
                                                bqs[:, 0:1])
                for i in range(2):
                    nc.vector.tensor_add(
                        vp[:, i, :, 0:HD],
                        pv01[i].rearrange("p (h d) -> p h d", h=4),
                        bvb.rearrange("p (h d) -> p h d", h=4))
                # V2..V9 reuse the freed pre-phase banks; they execute in
                # the window between xT fully landing and the exp stream
                # saturating, unloading the first attention block.
                for nb in range(2, 10):
                    pvn = pre.tile([128, E], F32, tag=f"p{(nb - 2) % 7}",
                                   name=f"pvn{nb}")
                    for dc in range(DC):
                        nc.tensor.matmul(
                            pvn, xap(dc, nb * 128, (nb + 1) * 128),
                            wv[dc],
                            start=(dc == 0), stop=(dc == DC - 1))
                    nc.vector.tensor_add(
                        vp[:, nb, :, 0:HD],
                        pvn.rearrange("p (h d) -> p h d", h=4),
                        bvb.rearrange("p (h d) -> p h d", h=4))

            with tc.tile_pool(name="ps", bufs=1, space="PSUM") as ps:
                pj_n = [0]

                def pj_tag():
                    pj_n[0] += 1
                    return ("pjA", "pjB")[pj_n[0] % 2]

                # ---- filler units (1-bank psum groups on pj tags) ----
                # K/Q projection groups run at boosted priority: they feed
                # the NEXT block's scores and must not queue behind the
                # V-projection / PV backlog.
                def proj_group(wsb, dst, bias, pair, n4):
                    def emit():
                        with tc.high_priority(offset=1 << 19):
                            pt = ps.tile([128, 512], F32, tag=pj_tag(),
                                         name="ppj")
                            for dc in range(DC):
                                nc.tensor.matmul(
                                    pt[:, :],
                                    wsb[dc][:, pair * 128:(pair + 1) * 128],
                                    xap(dc, n4 * 512, (n4 + 1) * 512),
                                    start=(dc == 0), stop=(dc == DC - 1))
                            nc.vector.tensor_scalar_add(
                                dst[:, pair, n4 * 512:(n4 + 1) * 512], pt[:, :],
                                bias[:, pair:pair + 1])
                    return emit

                def vnat_group(nb):
                    def emit():
                        pt = ps.tile([128, E], F32, tag=pj_tag(), name="pvn")
                        for dc in range(DC):
                            nc.tensor.matmul(
                                pt[:, :],
                                xap(dc, nb * 128, (nb + 1) * 128),
                                wv[dc],
                                start=(dc == 0), stop=(dc == DC - 1))
                        nc.vector.tensor_add(
                            vp[:, nb, :, 0:HD],
                            pt.rearrange("p (h d) -> p h d", h=4),
                            bvb.rearrange("p (h d) -> p h d", h=4))
                    return emit

                o_n = [0]

                def oproj_unit(nb, evict="dve", tags=None, ring=None):
                    # both D-halves of a 128-token block -> one contiguous
                    # [128, 1024] row store
                    def emit():
                        ot = op.tile([128, 1024], DT, tag="ot", name="ot")
                        pos = [ps.tile([128, 512], F32,
                                       tag=(tags[h] if tags else pj_tag()),
                                       name="po")
                               for h in range(2)]
                        # ec-outer: each at-chunk stationary serves both
                        # D-halves consecutively
                        for ec in range(2):
                            for half in range(2):
                                nc.tensor.matmul(
                                    pos[half][:, :],
                                    at[:, ec, nb * 128:(nb + 1) * 128],
                                    wo[:, ec, half * 512:(half + 1) * 512],
                                    start=(ec == 0), stop=(ec == 1))
                        for half in range(2):
                            if evict == "dve":
                                nc.vector.tensor_copy(
                                    ot[:, half * 512:(half + 1) * 512],
                                    pos[half])
                            else:
                                nc.scalar.copy(
                                    ot[:, half * 512:(half + 1) * 512],
                                    pos[half])
                        o_n[0] += 1
                        (ring or qs[o_n[0] % 2]).dma_start(
                            out=out[nb * 128:(nb + 1) * 128, :], in_=ot)
                    return emit

                # ---- one (pair, qc) attention block: 8 two-kb groups ----
                # Per group: 4 scores matmuls into one 4-bank PSUM tile, a
                # single [128, 2048] exp (amortizes the 352-cycle ACT
                # startup), then the previous group's 4 PV matmuls.
                # Returns a `finish` closure (last PV group + normalization);
                # the caller runs it after the NEXT block's first group so
                # ScalarE never stalls across block boundaries.
                def attn_block(pair, qc, fillers, carry=None,
                               pv_prio=0):
                    q0 = qc * QC
                    fi = 0
                    pvs = [ps.tile([HD + 1, QC], F32, tag=t, name=t)
                           for t in ("pvA", "pvB")]
                    wtiles = {}
                    for g in range(G):
                        st = ps.tile([128, 4, 512], F32, tag="st", name="st")
                        # scores for the two heads of both k-blocks
                        # back-to-back at max priority so they sit adjacent
                        # in the PE queue.
                        with tc.high_priority(offset=1 << 20):
                            for j in range(2):
                                kb = 2 * g + j
                                for hh in range(2):
                                    p0 = hh * HD
                                    nc.tensor.matmul(
                                        st[:, 2 * j + hh, :],
                                        kt[p0:p0 + HD, pair,
                                           kb * 128:(kb + 1) * 128],
                                        qt[p0:p0 + HD, pair, q0:q0 + QC],
                                        start=True, stop=True,
                                        tile_position=(p0, 0))
                        w = wp.tile([128, 4, 512], DT, tag="w", name="w")
                        nc.scalar.activation(w, st, AF.Exp, scale=SCALE)
                        wtiles[g] = w
                        if g == 0 and carry is not None:
                            carry()
                            carry = None
                        while fi < (g + 1) * len(fillers) // G:
                            fillers[fi]()
                            fi += 1
                        if g > 0:
                            wprev = wtiles.pop(g - 1)
                            with tc.high_priority(offset=pv_prio):
                                for j in range(2):
                                    kb = 2 * (g - 1) + j
                                    for hh in range(2):
                                        nc.tensor.matmul(
                                            pvs[hh][:, :],
                                            vp[:, kb, 2 * pair + hh, 0:HD + 1],
                                            wprev[:, 2 * j + hh, :],
                                            start=(kb == 0), stop=False)
                    wlast = wtiles.pop(G - 1)

                    def finish():
                        for j in range(2):
                            kb = 2 * (G - 1) + j
                            for hh in range(2):
                                nc.tensor.matmul(
                                    pvs[hh][:, :],
                                    vp[:, kb, 2 * pair + hh, 0:HD + 1],
                                    wlast[:, 2 * j + hh, :],
                                    start=False, stop=(kb == NB - 1))
                        # normalize both heads, ops interleaved so the GP
                        # broadcasts overlap the DVE chain; the at-mul reads
                        # pv straight from PSUM.  High priority: freeing the
                        # pv banks gates the next block's PV accumulation.
                        den = [dn.tile([1, QC], F32, tag=f"den{h}",
                                       name=f"den{h}") for h in range(2)]
                        rec = [dn.tile([1, QC], F32, tag=f"rec{h}",
                                       name=f"rec{h}") for h in range(2)]
                        bcr = [up.tile([HD, QC], F32, tag=f"bcr{h}",
                                       name=f"bcr{h}") for h in range(2)]
                        for hh in range(2):
                            nc.vector.tensor_copy(den[hh],
                                                  pvs[hh][HD:HD + 1, :])
                        for hh in range(2):
                            nc.vector.reciprocal_approx_fast(rec[hh],
                                                             den[hh])
                            nc.gpsimd.partition_broadcast(bcr[hh],
                                                          rec[hh][0:1, :])
                        for hh in range(2):
                            p0 = hh * HD
                            nc.vector.tensor_mul(
                                at[p0:p0 + HD, pair, q0:q0 + QC],
                                pvs[hh][0:HD, :], bcr[hh])
                    return finish

                # ---- filler inventory ----
                V = [vnat_group(i) for i in range(NB)]
                K1 = [proj_group(wk, kt, bks, 1, g) for g in range(4)]
                Q0 = [proj_group(wq, qt, bqs, 0, g) for g in range(4)]
                Q1 = [proj_group(wq, qt, bqs, 1, g) for g in range(4)]
                # tail units (qc=3) run after attention: they can rotate
                # over the freed attention PSUM banks and use ScalarE for
                # half the evictions.
                TT = [("pjA", "pjB"), ("pvA", "pvB")]
                tailq = [nc.sync, nc.gpsimd, nc.scalar, nc.sync]
                O = [[oproj_unit(4 * qc + nb,
                                 evict=("dve" if qc < 3 else
                                        ("dve", "act")[nb % 2]),
                                 tags=(None if qc < 3 else TT[nb % 2]),
                                 ring=(None if qc < 3 else tailq[nb]))
                      for nb in range(4)] for qc in range(NQC)]

                # V[nb] feeds PV at iter nb of the qc=0 blocks; keep a
                # 2-iter lead.  K1 g must land before block (1,0) iter 4g.
                sched = [
                    (0, 0, [K1[0], V[10], V[11], V[12],
                            V[13], V[14], V[15], Q1[0]]),
                    (1, 0, [K1[1], K1[2], K1[3], Q0[1]]),
                    (0, 1, [Q1[1]] + O[0][0:2]),
                    (1, 1, O[0][2:4] + [Q0[2]]),
                    (0, 2, [Q1[2]] + O[1][0:2]),
                    (1, 2, O[1][2:4] + [Q0[3]]),
                    (0, 3, [Q1[3]] + O[2][0:2]),
                    (1, 3, O[2][2:4]),
                ]
                fin = None
                for bi, (pair, qc, fillers) in enumerate(sched):
                    fin = attn_block(pair, qc, fillers, carry=fin,
                                     pv_prio=(1 << 19) if bi == len(sched) - 1
                                     else 0)
                fin()
                for g in O[3]:
                    g()
    return nc


_CACHE = {}


def _build():
    if "nc" not in _CACHE:
        nc = bacc.Bacc("TRN2", target_bir_lowering=False, debug=False)
        _emit(nc)
        nc.compile()
        _CACHE["nc"] = nc
    return _CACHE["nc"]


def _swiz(a):
    # [C*128, M] -> [128, C*M]: partition p gets row p of every 128-row chunk
    cm, m = a.shape
    c = cm // 128
    return np.ascontiguousarray(
        a.reshape(c, 128, m).transpose(1, 0, 2)).reshape(128, c * m)


def make_in_maps(x, Wq, bq, Wk, bk, Wv, bv, Wo, bo):
    import ml_dtypes
    f32 = np.float32
    bt = ml_dtypes.bfloat16
    ones_np = np.ones((128, NB, 4), bt)
    xTs = [_swiz(np.ascontiguousarray(np.asarray(x[b], dtype=f32).T).astype(bt))
           for b in range(B)]
    in_maps = []
    for c in range(8):
        b, r0 = c // 4, (c % 4) * E
        rows = slice(r0, r0 + E)
        in_maps.append({
            "xT": xTs[b],
            "wqT": _swiz(np.ascontiguousarray(np.asarray(Wq, f32)[rows].T).astype(bt)),
            "wkT": _swiz(np.ascontiguousarray(np.asarray(Wk, f32)[rows].T).astype(bt)),
            "wvT": _swiz(np.ascontiguousarray(np.asarray(Wv, f32)[rows].T).astype(bt)),
            "woT": _swiz(np.ascontiguousarray(np.asarray(Wo, f32)[:, rows].T).astype(bt)),
            "bq2": np.ascontiguousarray(np.asarray(bq, f32)[rows].reshape(2, 128).T),
            "bk2": np.ascontiguousarray(np.asarray(bk, f32)[rows].reshape(2, 128).T),
            "bv1": np.ascontiguousarray(np.asarray(bv, f32)[rows]),
            "vones": ones_np,
        })
    return in_maps


def kernel(x, Wq, bq, Wk, bk, Wv, bv, Wo, bo, _spmd_kwargs=None):
    nc = _build()
    in_maps = make_in_maps(x, Wq, bq, Wk, bk, Wv, bv, Wo, bo)
    res = run_bass_kernel_spmd(nc, in_maps, core_ids=list(range(8)),
                               **(_spmd_kwargs or {}))
    parts = np.stack([np.asarray(res.results[c]["out"], np.float32)
                      for c in range(8)])
    outv = parts.reshape(B, 4, N, D).sum(axis=1) + np.asarray(bo, np.float32)
    if _spmd_kwargs:
        _CACHE["last_results"] = res
    return outv.astype(np.float32)


# revision 12
# speedup vs baseline: 1.0686x; 1.0686x over previous
"""Multi-head attention (B=2, N=2048, D=1024, H=16) on 8 Trainium2 cores.

Sharding: data-parallel over batch (cores 0-3 -> b=0, cores 4-7 -> b=1) and
tensor-parallel over heads (4 heads per core = 256 of 1024 QKV/O channels).
Each core computes its 4 heads' attention plus a partial output projection;
the host sums the 4 partials per batch and adds bo.

v3a pipeline (per core): baseline attention structure + need-ordered
chunked input DMA on the 3 rings so the first K-proj matmul fires ~9us in,
and ScalarE keeps its ring free during attention (stores ride sync/gpsimd).
"""

import numpy as np

import concourse.bass as bass
import concourse.bacc as bacc
import concourse.tile as tile
from concourse import mybir
from concourse.bass_utils import run_bass_kernel_spmd

F32 = mybir.dt.float32
BF16 = mybir.dt.bfloat16
AF = mybir.ActivationFunctionType

B, N, D, H, HD = 2, 2048, 1024, 16, 64
E = 256            # channels per core (4 heads * 64)
DC = D // 128      # 8 contraction chunks for projections
NB = N // 128      # 16 token blocks / k chunks
G = NB // 2        # 8 two-k-block groups per attention block
QC = 512           # query chunk
NQC = N // QC      # 4 query chunks
SCALE = 1.0 / np.sqrt(HD)
DT = BF16


def _emit(nc):
    # all big inputs arrive host-swizzled: per-partition contiguous rows so
    # the DMA descriptors are large (partition p owns columns p of every
    # 128-row chunk)
    xT = nc.dram_tensor("xT", [128, DC * N], DT, kind="ExternalInput")
    wqT = nc.dram_tensor("wqT", [128, DC * E], DT, kind="ExternalInput")
    wkT = nc.dram_tensor("wkT", [128, DC * E], DT, kind="ExternalInput")
    wvT = nc.dram_tensor("wvT", [128, DC * E], DT, kind="ExternalInput")
    woT = nc.dram_tensor("woT", [128, 2 * D], DT, kind="ExternalInput")
    bq2 = nc.dram_tensor("bq2", [128, 2], F32, kind="ExternalInput")
    bk2 = nc.dram_tensor("bk2", [128, 2], F32, kind="ExternalInput")
    bv1 = nc.dram_tensor("bv1", [E], F32, kind="ExternalInput")
    vones = nc.dram_tensor("vones", [128, NB, 4], DT, kind="ExternalInput")
    out = nc.dram_tensor("out", [N, D], DT, kind="ExternalOutput")

    with tile.TileContext(nc) as tc:
        with tc.tile_pool(name="per", bufs=1) as per, \
             tc.tile_pool(name="wp", bufs=6) as wp, \
             tc.tile_pool(name="dn", bufs=2) as dn, \
             tc.tile_pool(name="up", bufs=2) as up, \
             tc.tile_pool(name="op", bufs=6) as op:

            # ---- persistent SBUF tiles ----
            # x arrives as per-d-chunk tiles (chunk 0 split in half) so each
            # chunk's projection matmuls fire as soon as its DMA lands;
            # weights arrive as 4-chunk halves.  Separate tiles per DMA so
            # no false write-order deps serialize them.
            xt0 = [per.tile([128, N // 2], DT, name=f"xt0{i}") for i in range(2)]
            xts = [per.tile([128, N], DT, name=f"xt{i}") for i in range(1, DC)]

            def xap(dc, lo, hi):
                if dc == 0:
                    half = N // 2
                    if hi <= half:
                        return xt0[0][:, lo:hi]
                    if lo >= half:
                        return xt0[1][:, lo - half:hi - half]
                    raise AssertionError("x chunk-0 slice crosses halves")
                return xts[dc - 1][:, lo:hi]

            wq2 = [per.tile([128, 4, E], DT, name=f"wq2{i}") for i in range(2)]
            wk2 = [per.tile([128, 4, E], DT, name=f"wk2{i}") for i in range(2)]
            wv2 = [per.tile([128, 4, E], DT, name=f"wv2{i}") for i in range(2)]
            wq = [wq2[dc // 4][:, dc % 4, :] for dc in range(DC)]
            wk = [wk2[dc // 4][:, dc % 4, :] for dc in range(DC)]
            wv = [wv2[dc // 4][:, dc % 4, :] for dc in range(DC)]
            wo = per.tile([128, 2, D], DT)            # WoT (e-chunk)
            qt = per.tile([128, 2, N], DT)            # Q^T: (pair, tokens)
            kt = per.tile([128, 2, N], DT)
            vp = per.tile([128, NB, 4, 128], DT)      # V natural + ones col
            at = per.tile([128, 2, N], DT)            # attn^T normalized
            bqs = per.tile([128, 2], F32)
            bks = per.tile([128, 2], F32)
            bvb = per.tile([128, E], F32)

            qs = [nc.sync, nc.gpsimd]
            # ---- input DMA: need-ordered chunked transfers on 3 rings ----
            # Only sync/scalar/gpsimd can drive DMA.  Each ring's first
            # transfers are the ones that gate the pre-phase: wk chunk-half
            # + x chunk 0 halves land first so the first K-proj matmul
            # fires ~9us in; x chunks are interleaved so they arrive
            # roughly in d-chunk order.
            def wslice(dram, i):
                return dram[:, 4 * i * E:4 * (i + 1) * E].rearrange(
                    "p (c e) -> p c e", e=E)

            bv_ap = bv1[:]
            # sync ring
            nc.sync.dma_start(out=wk2[0], in_=wslice(wkT, 0))
            nc.sync.dma_start(out=xt0[0], in_=xT[:, 0:N // 2])
            nc.sync.dma_start(out=wk2[1], in_=wslice(wkT, 1))
            nc.sync.dma_start(out=bqs, in_=bq2[:, :])
            nc.sync.dma_start(out=bks, in_=bk2[:, :])
            nc.sync.dma_start(out=xts[1], in_=xT[:, 2 * N:3 * N])
            nc.sync.dma_start(out=xts[4], in_=xT[:, 5 * N:6 * N])
            # scalar ring (free until the first exp ~25us in)
            nc.scalar.dma_start(out=wq2[0], in_=wslice(wqT, 0))
            nc.scalar.dma_start(out=xt0[1], in_=xT[:, N // 2:N])
            nc.scalar.dma_start(out=xts[2], in_=xT[:, 3 * N:4 * N])
            nc.scalar.dma_start(out=xts[5], in_=xT[:, 6 * N:7 * N])
            nc.scalar.dma_start(out=wq2[1], in_=wslice(wqT, 1))
            nc.scalar.dma_start(out=vp[:, :, :, HD:HD + 1],
                                in_=vones[:, :, :].rearrange(
                                    "p a (b o) -> p a b o", o=1))
            nc.scalar.dma_start(out=wo, in_=woT.rearrange(
                "p (c e) -> p c e", e=D))
            # gpsimd ring
            nc.gpsimd.dma_start(out=wv2[0], in_=wslice(wvT, 0))
            nc.gpsimd.dma_start(out=wv2[1], in_=wslice(wvT, 1))
            nc.gpsimd.dma_start(
                out=bvb,
                in_=bass.AP(tensor=bv_ap.tensor, offset=0, ap=[[0, 128], [1, E]]),
            )
            nc.gpsimd.dma_start(out=xts[0], in_=xT[:, N:2 * N])
            nc.gpsimd.dma_start(out=xts[3], in_=xT[:, 4 * N:5 * N])
            nc.gpsimd.dma_start(out=xts[6], in_=xT[:, 7 * N:8 * N])

            # ---- pre-phase: warmup + chunk-major first projections ----
            # K0 g0-3, Q0 g0, V nb0-1 accumulate concurrently in a dedicated
            # PSUM pool (7 banks); each d-chunk's matmuls fire as the chunk
            # arrives from HBM.
            with tc.tile_pool(name="pre", bufs=1, space="PSUM") as pre:
                pk = [pre.tile([128, 512], F32, tag=f"p{g}", name=f"pk{g}")
                      for g in range(4)]
                pq0 = pre.tile([128, 512], F32, tag="p4", name="pq0")
                pv01 = [pre.tile([128, E], F32, tag=f"p{5 + i}", name=f"pv{i}")
                        for i in range(2)]
                for dc in range(DC):
                    for g in range(4):
                        nc.tensor.matmul(
                            pk[g], wk[dc][:, 0:128],
                            xap(dc, g * 512, (g + 1) * 512),
                            start=(dc == 0), stop=(dc == DC - 1))
                    nc.tensor.matmul(
                        pq0, wq[dc][:, 0:128], xap(dc, 0, 512),
                        start=(dc == 0), stop=(dc == DC - 1))
                    for i in range(2):
                        nc.tensor.matmul(
                            pv01[i], xap(dc, i * 128, (i + 1) * 128),
                            wv[dc],
                            start=(dc == 0), stop=(dc == DC - 1))
                with tc.high_priority(offset=1 << 19):
                    for g in range(4):
                        nc.vector.tensor_scalar_add(
                            kt[:, 0, g * 512:(g + 1) * 512], pk[g],
                            bks[:, 0:1])
                    nc.vector.tensor_scalar_add(qt[:, 0, 0:512], pq0,
                                                bqs[:, 0:1])
                for i in range(2):
                    nc.vector.tensor_add(
                        vp[:, i, :, 0:HD],
                        pv01[i].rearrange("p (h d) -> p h d", h=4),
                        bvb.rearrange("p (h d) -> p h d", h=4))
                # V2..V9 reuse the freed pre-phase banks; they execute in
                # the window between xT fully landing and the exp stream
                # saturating, unloading the first attention block.
                for nb in range(2, 10):
                    pvn = pre.tile([128, E], F32, tag=f"p{(nb - 2) % 7}",
                                   name=f"pvn{nb}")
                    for dc in range(DC):
                        nc.tensor.matmul(
                            pvn, xap(dc, nb * 128, (nb + 1) * 128),
                            wv[dc],
                            start=(dc == 0), stop=(dc == DC - 1))
                    nc.vector.tensor_add(
                        vp[:, nb, :, 0:HD],
                        pvn.rearrange("p (h d) -> p h d", h=4),
                        bvb.rearrange("p (h d) -> p h d", h=4))

            with tc.tile_pool(name="ps", bufs=1, space="PSUM") as ps:
                pj_n = [0]

                def pj_tag():
                    pj_n[0] += 1
                    return ("pjA", "pjB")[pj_n[0] % 2]

                # ---- filler units (1-bank psum groups on pj tags) ----
                # K/Q projection groups run at boosted priority: they feed
                # the NEXT block's scores and must not queue behind the
                # V-projection / PV backlog.
                def proj_group(wsb, dst, bias, pair, n4):
                    def emit():
                        with tc.high_priority(offset=1 << 19):
                            pt = ps.tile([128, 512], F32, tag=pj_tag(),
                                         name="ppj")
                            for dc in range(DC):
                                nc.tensor.matmul(
                                    pt[:, :],
                                    wsb[dc][:, pair * 128:(pair + 1) * 128],
                                    xap(dc, n4 * 512, (n4 + 1) * 512),
                                    start=(dc == 0), stop=(dc == DC - 1))
                            nc.vector.tensor_scalar_add(
                                dst[:, pair, n4 * 512:(n4 + 1) * 512], pt[:, :],
                                bias[:, pair:pair + 1])
                    return emit

                def vnat_group(nb):
                    def emit():
                        pt = ps.tile([128, E], F32, tag=pj_tag(), name="pvn")
                        for dc in range(DC):
                            nc.tensor.matmul(
                                pt[:, :],
                                xap(dc, nb * 128, (nb + 1) * 128),
                                wv[dc],
                                start=(dc == 0), stop=(dc == DC - 1))
                        nc.vector.tensor_add(
                            vp[:, nb, :, 0:HD],
                            pt.rearrange("p (h d) -> p h d", h=4),
                            bvb.rearrange("p (h d) -> p h d", h=4))
                    return emit

                o_n = [0]

                def oproj_unit(nb, evict="dve", tags=None, ring=None):
                    # both D-halves of a 128-token block -> one contiguous
                    # [128, 1024] row store
                    def emit():
                        ot = op.tile([128, 1024], DT, tag="ot", name="ot")
                        pos = [ps.tile([128, 512], F32,
                                       tag=(tags[h] if tags else pj_tag()),
                                       name="po")
                               for h in range(2)]
                        # ec-outer: each at-chunk stationary serves both
                        # D-halves consecutively
                        for ec in range(2):
                            for half in range(2):
                                nc.tensor.matmul(
                                    pos[half][:, :],
                                    at[:, ec, nb * 128:(nb + 1) * 128],
                                    wo[:, ec, half * 512:(half + 1) * 512],
                                    start=(ec == 0), stop=(ec == 1))
                        for half in range(2):
                            if evict == "dve":
                                nc.vector.tensor_copy(
                                    ot[:, half * 512:(half + 1) * 512],
                                    pos[half])
                            else:
                                nc.scalar.copy(
                                    ot[:, half * 512:(half + 1) * 512],
                                    pos[half])
                        o_n[0] += 1
                        (ring or qs[o_n[0] % 2]).dma_start(
                            out=out[nb * 128:(nb + 1) * 128, :], in_=ot)
                    return emit

                # ---- one (pair, qc) attention block: 8 two-kb groups ----
                # Per group: 4 scores matmuls into one 4-bank PSUM tile, a
                # single [128, 2048] exp (amortizes the 352-cycle ACT
                # startup), then the previous group's 4 PV matmuls.
                # Returns a `finish` closure (last PV group + normalization);
                # the caller runs it after the NEXT block's first group so
                # ScalarE never stalls across block boundaries.
                def attn_block(pair, qc, fillers, carry=None,
                               pv_prio=0):
                    q0 = qc * QC
                    fi = 0
                    pvs = [ps.tile([HD + 1, QC], F32, tag=t, name=t)
                           for t in ("pvA", "pvB")]
                    wtiles = {}
                    for g in range(G):
                        st = ps.tile([128, 4, 512], F32, tag="st", name="st")
                        # scores at default priority: with a single 4-bank
                        # st buffer, scores(g+1) must sit BEHIND PV(g-1) and
                        # the fillers in the PE FIFO, or it head-of-line
                        # blocks on exp(g) freeing the buffer.
                        for j in range(2):
                            kb = 2 * g + j
                            for hh in range(2):
                                p0 = hh * HD
                                nc.tensor.matmul(
                                    st[:, 2 * j + hh, :],
                                    kt[p0:p0 + HD, pair,
                                       kb * 128:(kb + 1) * 128],
                                    qt[p0:p0 + HD, pair, q0:q0 + QC],
                                    start=True, stop=True,
                                    tile_position=(p0, 0))
                        w = wp.tile([128, 4, 512], DT, tag="w", name="w")
                        nc.scalar.activation(w, st, AF.Exp, scale=SCALE)
                        wtiles[g] = w
                        if g == 0 and carry is not None:
                            carry()
                            carry = None
                        while fi < (g + 1) * len(fillers) // G:
                            fillers[fi]()
                            fi += 1
                        if g > 0:
                            wprev = wtiles.pop(g - 1)
                            with tc.high_priority(offset=pv_prio):
                                for j in range(2):
                                    kb = 2 * (g - 1) + j
                                    for hh in range(2):
                                        nc.tensor.matmul(
                                            pvs[hh][:, :],
                                            vp[:, kb, 2 * pair + hh, 0:HD + 1],
                                            wprev[:, 2 * j + hh, :],
                                            start=(kb == 0), stop=False)
                    wlast = wtiles.pop(G - 1)

                    def finish():
                        for j in range(2):
                            kb = 2 * (G - 1) + j
                            for hh in range(2):
                                nc.tensor.matmul(
                                    pvs[hh][:, :],
                                    vp[:, kb, 2 * pair + hh, 0:HD + 1],
                                    wlast[:, 2 * j + hh, :],
                                    start=False, stop=(kb == NB - 1))
                        # normalize both heads, ops interleaved so the GP
                        # broadcasts overlap the DVE chain; the at-mul reads
                        # pv straight from PSUM.  High priority: freeing the
                        # pv banks gates the next block's PV accumulation.
                        den = [dn.tile([1, QC], F32, tag=f"den{h}",
                                       name=f"den{h}") for h in range(2)]
                        rec = [dn.tile([1, QC], F32, tag=f"rec{h}",
                                       name=f"rec{h}") for h in range(2)]
                        bcr = [up.tile([HD, QC], F32, tag=f"bcr{h}",
                                       name=f"bcr{h}") for h in range(2)]
                        for hh in range(2):
                            nc.vector.tensor_copy(den[hh],
                                                  pvs[hh][HD:HD + 1, :])
                        for hh in range(2):
                            nc.vector.reciprocal_approx_fast(rec[hh],
                                                             den[hh])
                            nc.gpsimd.partition_broadcast(bcr[hh],
                                                          rec[hh][0:1, :])
                        for hh in range(2):
                            p0 = hh * HD
                            nc.vector.tensor_mul(
                                at[p0:p0 + HD, pair, q0:q0 + QC],
                                pvs[hh][0:HD, :], bcr[hh])
                    return finish

                # ---- filler inventory ----
                V = [vnat_group(i) for i in range(NB)]
                K1 = [proj_group(wk, kt, bks, 1, g) for g in range(4)]
                Q0 = [proj_group(wq, qt, bqs, 0, g) for g in range(4)]
                Q1 = [proj_group(wq, qt, bqs, 1, g) for g in range(4)]
                # tail units (qc=3) run after attention: they can rotate
                # over the freed attention PSUM banks and use ScalarE for
                # half the evictions.
                TT = [("pjA", "pjB"), ("pvA", "pvB")]
                tailq = [nc.sync, nc.gpsimd, nc.scalar, nc.sync]
                O = [[oproj_unit(4 * qc + nb,
                                 evict=("dve" if qc < 3 else
                                        ("dve", "act")[nb % 2]),
                                 tags=(None if qc < 3 else TT[nb % 2]),
                                 ring=(None if qc < 3 else tailq[nb]))
                      for nb in range(4)] for qc in range(NQC)]

                # V[nb] feeds PV at iter nb of the qc=0 blocks; keep a
                # 2-iter lead.  K1 g must land before block (1,0) iter 4g.
                sched = [
                    (0, 0, [K1[0], V[10], V[11], V[12],
                            V[13], V[14], V[15], Q1[0]]),
                    (1, 0, [K1[1], K1[2], K1[3], Q0[1]]),
                    (0, 1, [Q1[1]] + O[0][0:2]),
                    (1, 1, O[0][2:4] + [Q0[2]]),
                    (0, 2, [Q1[2]] + O[1][0:2]),
                    (1, 2, O[1][2:4] + [Q0[3]]),
                    (0, 3, [Q1[3]] + O[2][0:2]),
                    (1, 3, O[2][2:4]),
                ]
                fin = None
                for bi, (pair, qc, fillers) in enumerate(sched):
                    fin = attn_block(pair, qc, fillers, carry=fin,
                                     pv_prio=(1 << 19) if bi == len(sched) - 1
                                     else 0)
                fin()
                for g in O[3]:
                    g()
    return nc


_CACHE = {}


def _build():
    if "nc" not in _CACHE:
        nc = bacc.Bacc("TRN2", target_bir_lowering=False, debug=False)
        _emit(nc)
        nc.compile()
        _CACHE["nc"] = nc
    return _CACHE["nc"]


def _swiz(a):
    # [C*128, M] -> [128, C*M]: partition p gets row p of every 128-row chunk
    cm, m = a.shape
    c = cm // 128
    return np.ascontiguousarray(
        a.reshape(c, 128, m).transpose(1, 0, 2)).reshape(128, c * m)


def make_in_maps(x, Wq, bq, Wk, bk, Wv, bv, Wo, bo):
    import ml_dtypes
    f32 = np.float32
    bt = ml_dtypes.bfloat16
    ones_np = np.ones((128, NB, 4), bt)
    xTs = [_swiz(np.ascontiguousarray(np.asarray(x[b], dtype=f32).T).astype(bt))
           for b in range(B)]
    in_maps = []
    for c in range(8):
        b, r0 = c // 4, (c % 4) * E
        rows = slice(r0, r0 + E)
        in_maps.append({
            "xT": xTs[b],
            "wqT": _swiz(np.ascontiguousarray(np.asarray(Wq, f32)[rows].T).astype(bt)),
            "wkT": _swiz(np.ascontiguousarray(np.asarray(Wk, f32)[rows].T).astype(bt)),
            "wvT": _swiz(np.ascontiguousarray(np.asarray(Wv, f32)[rows].T).astype(bt)),
            "woT": _swiz(np.ascontiguousarray(np.asarray(Wo, f32)[:, rows].T).astype(bt)),
            "bq2": np.ascontiguousarray(np.asarray(bq, f32)[rows].reshape(2, 128).T),
            "bk2": np.ascontiguousarray(np.asarray(bk, f32)[rows].reshape(2, 128).T),
            "bv1": np.ascontiguousarray(np.asarray(bv, f32)[rows]),
            "vones": ones_np,
        })
    return in_maps


def kernel(x, Wq, bq, Wk, bk, Wv, bv, Wo, bo, _spmd_kwargs=None):
    nc = _build()
    in_maps = make_in_maps(x, Wq, bq, Wk, bk, Wv, bv, Wo, bo)
    res = run_bass_kernel_spmd(nc, in_maps, core_ids=list(range(8)),
                               **(_spmd_kwargs or {}))
    parts = np.stack([np.asarray(res.results[c]["out"], np.float32)
                      for c in range(8)])
    outv = parts.reshape(B, 4, N, D).sum(axis=1) + np.asarray(bo, np.float32)
    if _spmd_kwargs:
        _CACHE["last_results"] = res
    return outv.astype(np.float32)


# revision 15
# speedup vs baseline: 1.3493x; 1.2626x over previous
"""Multi-head attention (B=2, N=2048, D=1024, H=16) on 8 Trainium2 cores.

Sharding: data-parallel over batch (cores 0-3 -> b=0, cores 4-7 -> b=1) and
tensor-parallel over heads (4 heads per core = 256 of 1024 QKV/O channels).
Each core computes its 4 heads' attention plus a partial output projection;
the host sums the 4 partials per batch and adds bo.

v4 pipeline (per core): the kernel is exp-chain bound (16.8M softmax exps
must stream through ScalarE's 128 lanes at ~1.15us per key-block), so the
win is starting that chain early and never stalling it.
 - x arrives TOKEN-MAJOR: the host lays xT out as (token-group, d-chunk)
   so K/Q projections for tokens 0-511 complete after ~1.5MB of DMA and
   the first scores+exp fire ~15us in, ~14us earlier than a d-chunk-major
   stream.  Remaining x token-groups stream in behind the running
   attention; the K projections for later token-groups and all V/Q/K-pair1
   projections run as PE filler units pinned to specific k-iterations of
   the first two attention blocks, paced to match their DMA arrival.
 - Attention processes a (pair, 512-query-chunk) block at a time.  Per
   k-iter the TWO heads of the pair run their scores matmuls CONCURRENTLY
   in disjoint PE row-groups (K=64 each, tile_position (0,0)/(64,0)) into
   the two halves of one [128,1024] PSUM tile; a single FD=1024 exp on
   ScalarE covers both heads; PV (M=65 with the ones/denominator column)
   runs per head with a one-iter lag.
 - ScalarE's DMA ring is used only during the intro (it is exp-bound
   after); output stores ride sync/gpsimd, plus scalar for the tail.
 - Normalization: den copy, reciprocal_approx_fast, GPSIMD
   partition_broadcast, one tensor_mul reading pv straight from PSUM.
"""

import numpy as np

import concourse.bass as bass
import concourse.bacc as bacc
import concourse.tile as tile
from concourse import mybir
from concourse.bass_utils import run_bass_kernel_spmd

F32 = mybir.dt.float32
BF16 = mybir.dt.bfloat16
AF = mybir.ActivationFunctionType

B, N, D, H, HD = 2, 2048, 1024, 16, 64
E = 256            # channels per core (4 heads * 64)
DC = D // 128      # 8 contraction chunks for projections
NB = N // 128      # 16 token blocks / k chunks
TG = 4             # 512-token groups (x DMA + K/Q projection granularity)
QC = 512           # query chunk
NQC = N // QC      # 4 query chunks
SCALE = 1.0 / np.sqrt(HD)
DT = BF16


def _emit(nc):
    # all big inputs arrive host-swizzled: per-partition contiguous rows so
    # the DMA descriptors are large (partition p owns columns p of every
    # 128-row chunk).  xT is token-group-major: [128, (tg, dc, 512)].
    # wqT/wkT are pair-major: [128, (pair, dc, 128)].
    xT = nc.dram_tensor("xT", [128, DC * N], DT, kind="ExternalInput")
    wqT = nc.dram_tensor("wqT", [128, DC * E], DT, kind="ExternalInput")
    wkT = nc.dram_tensor("wkT", [128, DC * E], DT, kind="ExternalInput")
    wvT = nc.dram_tensor("wvT", [128, DC * E], DT, kind="ExternalInput")
    woT = nc.dram_tensor("woT", [128, 2 * D], DT, kind="ExternalInput")
    bq2 = nc.dram_tensor("bq2", [128, 2], F32, kind="ExternalInput")
    bk2 = nc.dram_tensor("bk2", [128, 2], F32, kind="ExternalInput")
    bv1 = nc.dram_tensor("bv1", [E], F32, kind="ExternalInput")
    vones = nc.dram_tensor("vones", [128, NB, 4], DT, kind="ExternalInput")
    out = nc.dram_tensor("out", [N, D], DT, kind="ExternalOutput")

    with tile.TileContext(nc) as tc:
        with tc.tile_pool(name="per", bufs=1) as per, \
             tc.tile_pool(name="wp", bufs=12) as wp, \
             tc.tile_pool(name="dn", bufs=2) as dn, \
             tc.tile_pool(name="up", bufs=2) as up, \
             tc.tile_pool(name="op", bufs=6) as op:

            # ---- persistent SBUF tiles ----
            # x arrives as (token-group, d-chunk-piece) tiles: pieces A/B/C
            # hold d-chunks 0-2 / 3-5 / 6-7 of one 512-token group, each a
            # single contiguous DMA.  Separate tiles per DMA so no false
            # write-order deps serialize them.
            xgA = [per.tile([128, 3, 512], DT, name=f"xgA{t}") for t in range(TG)]
            xgB = [per.tile([128, 3, 512], DT, name=f"xgB{t}") for t in range(TG)]
            xgC = [per.tile([128, 2, 512], DT, name=f"xgC{t}") for t in range(TG)]

            def xap(dc, lo, hi):
                tg, o, oh = lo // 512, lo % 512, hi - (lo // 512) * 512
                assert oh <= 512, "x slice crosses token groups"
                if dc < 3:
                    return xgA[tg][:, dc, o:oh]
                if dc < 6:
                    return xgB[tg][:, dc - 3, o:oh]
                return xgC[tg][:, dc - 6, o:oh]

            wqp = [per.tile([128, DC, 128], DT, name=f"wqp{i}") for i in range(2)]
            wkp = [per.tile([128, DC, 128], DT, name=f"wkp{i}") for i in range(2)]
            wv2 = [per.tile([128, 4, E], DT, name=f"wv2{i}") for i in range(2)]
            wv = [wv2[dc // 4][:, dc % 4, :] for dc in range(DC)]
            wo = per.tile([128, 2, D], DT)            # WoT (e-chunk)
            qt = per.tile([128, 2, N], DT)            # Q^T: (pair, tokens)
            kt = per.tile([128, 2, N], DT)
            vp = per.tile([128, NB, 4, 128], DT)      # V natural + ones col
            at = per.tile([128, 2, N], DT)            # attn^T normalized
            bqs = per.tile([128, 2], F32)
            bks = per.tile([128, 2], F32)
            bvb = per.tile([128, E], F32)

            qs = [nc.sync, nc.gpsimd]
            # ---- input DMA: need-ordered transfers on the 3 rings ----
            # Deadlines (at ~330GB/s aggregate): wk/wq pair0 + x tg0 by
            # ~13us (attention start), wv by ~15, x tg1 by ~18, x tg2 by
            # ~21, wk pair1 by ~23, x tg3 by ~25, wq pair1 / wo later.
            def xpiece(tg, lo, hi):
                return xT[:, tg * DC * 512 + lo * 512:
                          tg * DC * 512 + hi * 512].rearrange(
                    "p (c n) -> p c n", n=512)

            def wpair(dram, p):
                return dram[:, p * DC * 128:(p + 1) * DC * 128].rearrange(
                    "p (c e) -> p c e", e=128)

            bv_ap = bv1[:]
            # sync ring
            nc.sync.dma_start(out=wkp[0], in_=wpair(wkT, 0))
            nc.sync.dma_start(out=bqs, in_=bq2[:, :])
            nc.sync.dma_start(out=bks, in_=bk2[:, :])
            nc.sync.dma_start(out=xgA[0], in_=xpiece(0, 0, 3))
            nc.sync.dma_start(out=wv2[0], in_=wvT[:, 0:4 * E].rearrange(
                "p (c e) -> p c e", e=E))
            nc.sync.dma_start(out=xgA[1], in_=xpiece(1, 0, 3))
            nc.sync.dma_start(out=xgA[2], in_=xpiece(2, 0, 3))
            nc.sync.dma_start(out=xgA[3], in_=xpiece(3, 0, 3))
            # scalar ring (free until the first exp ~15us in, so it only
            # carries pieces needed by then; wq pair1/vones ride the tail)
            nc.scalar.dma_start(out=wqp[0], in_=wpair(wqT, 0))
            nc.scalar.dma_start(out=xgB[0], in_=xpiece(0, 3, 6))
            nc.scalar.dma_start(out=wv2[1], in_=wvT[:, 4 * E:8 * E].rearrange(
                "p (c e) -> p c e", e=E))
            nc.scalar.dma_start(out=xgB[1], in_=xpiece(1, 3, 6))
            nc.scalar.dma_start(out=xgB[2], in_=xpiece(2, 3, 6))
            nc.scalar.dma_start(out=xgB[3], in_=xpiece(3, 3, 6))
            nc.scalar.dma_start(out=wqp[1], in_=wpair(wqT, 1))
            nc.scalar.dma_start(out=vp[:, :, :, HD:HD + 1],
                                in_=vones[:, :, :].rearrange(
                                    "p a (b o) -> p a b o", o=1))
            # gpsimd ring
            nc.gpsimd.dma_start(
                out=bvb,
                in_=bass.AP(tensor=bv_ap.tensor, offset=0, ap=[[0, 128], [1, E]]),
            )
            nc.gpsimd.dma_start(out=xgC[0], in_=xpiece(0, 6, 8))
            nc.gpsimd.dma_start(out=xgC[1], in_=xpiece(1, 6, 8))
            nc.gpsimd.dma_start(out=xgC[2], in_=xpiece(2, 6, 8))
            nc.gpsimd.dma_start(out=wkp[1], in_=wpair(wkT, 1))
            nc.gpsimd.dma_start(out=xgC[3], in_=xpiece(3, 6, 8))
            nc.gpsimd.dma_start(out=wo, in_=woT.rearrange(
                "p (c e) -> p c e", e=D))

            # ---- pre-phase: K pair0 tg0 + Q pair0 g0 only ----
            # (everything else runs as pinned/paced fillers inside the
            # attention blocks, matched to DMA arrival order)
            with tc.tile_pool(name="pre", bufs=1, space="PSUM") as pre:
                pk0 = pre.tile([128, 512], F32, tag="p0", name="pk0")
                pq0 = pre.tile([128, 512], F32, tag="p1", name="pq0")
                for dc in range(DC):
                    nc.tensor.matmul(
                        pk0, wkp[0][:, dc, :], xap(dc, 0, 512),
                        start=(dc == 0), stop=(dc == DC - 1))
                for dc in range(DC):
                    nc.tensor.matmul(
                        pq0, wqp[0][:, dc, :], xap(dc, 0, 512),
                        start=(dc == 0), stop=(dc == DC - 1))
                with tc.high_priority(offset=1 << 19):
                    nc.vector.tensor_scalar_add(kt[:, 0, 0:512], pk0,
                                                bks[:, 0:1])
                    nc.vector.tensor_scalar_add(qt[:, 0, 0:512], pq0,
                                                bqs[:, 0:1])

            with tc.tile_pool(name="ps", bufs=1, space="PSUM") as ps:
                pj_n = [0]

                def pj_tag():
                    pj_n[0] += 1
                    return ("pjA", "pjB")[pj_n[0] % 2]

                # ---- filler units (1-bank psum groups on pj tags) ----
                # K/Q projection groups run at boosted priority: they feed
                # upcoming scores and must not queue behind the
                # V-projection / PV backlog.
                def proj_group(wpt, dst, bias, pair, n4):
                    def emit():
                        with tc.high_priority(offset=1 << 19):
                            pt = ps.tile([128, 512], F32, tag=pj_tag(),
                                         name="ppj")
                            for dc in range(DC):
                                nc.tensor.matmul(
                                    pt[:, :],
                                    wpt[:, dc, :],
                                    xap(dc, n4 * 512, (n4 + 1) * 512),
                                    start=(dc == 0), stop=(dc == DC - 1))
                            nc.vector.tensor_scalar_add(
                                dst[:, pair, n4 * 512:(n4 + 1) * 512], pt[:, :],
                                bias[:, pair:pair + 1])
                    return emit

                def vnat_group(nb):
                    def emit():
                        pt = ps.tile([128, E], F32, tag=pj_tag(), name="pvn")
                        for dc in range(DC):
                            nc.tensor.matmul(
                                pt[:, :],
                                xap(dc, nb * 128, (nb + 1) * 128),
                                wv[dc],
                                start=(dc == 0), stop=(dc == DC - 1))
                        nc.vector.tensor_add(
                            vp[:, nb, :, 0:HD],
                            pt.rearrange("p (h d) -> p h d", h=4),
                            bvb.rearrange("p (h d) -> p h d", h=4))
                    return emit

                o_n = [0]

                def oproj_unit(nb, evict="dve", tags=None, ring=None):
                    # both D-halves of a 128-token block -> one contiguous
                    # [128, 1024] row store
                    def emit():
                        ot = op.tile([128, 1024], DT, tag="ot", name="ot")
                        pos = [ps.tile([128, 512], F32,
                                       tag=(tags[h] if tags else pj_tag()),
                                       name="po")
                               for h in range(2)]
                        # ec-outer: each at-chunk stationary serves both
                        # D-halves consecutively
                        for ec in range(2):
                            for half in range(2):
                                nc.tensor.matmul(
                                    pos[half][:, :],
                                    at[:, ec, nb * 128:(nb + 1) * 128],
                                    wo[:, ec, half * 512:(half + 1) * 512],
                                    start=(ec == 0), stop=(ec == 1))
                        for half in range(2):
                            if evict == "dve":
                                nc.vector.tensor_copy(
                                    ot[:, half * 512:(half + 1) * 512],
                                    pos[half])
                            else:
                                nc.scalar.copy(
                                    ot[:, half * 512:(half + 1) * 512],
                                    pos[half])
                        o_n[0] += 1
                        (ring or qs[o_n[0] % 2]).dma_start(
                            out=out[nb * 128:(nb + 1) * 128, :], in_=ot)
                    return emit

                # ---- one (pair, qc) attention block: 16 k-iters ----
                # Returns a `finish` closure (last PV pair + normalization);
                # the caller runs it after the NEXT block's first k-iter so
                # ScalarE never stalls across block boundaries.  `pinned`
                # maps k-iter -> filler units that must run at that iter
                # (used to match DMA arrival order in the first blocks).
                def attn_block(pair, qc, fillers, pinned=None, carry=None,
                               pv_prio=0):
                    q0 = qc * QC
                    fi = 0
                    pvs = [ps.tile([HD + 1, QC], F32, tag=t, name=t)
                           for t in ("pvA", "pvB")]
                    wtiles = {}
                    for k in range(NB):
                        st = ps.tile([128, 1024], F32,
                                     tag=("s0", "s1")[k % 2], name="st")
                        # scores for both heads back-to-back at max priority
                        # so they sit adjacent in the PE queue and overlap in
                        # disjoint row-groups of the array.
                        with tc.high_priority(offset=1 << 20):
                            for hh in range(2):
                                p0 = hh * HD
                                nc.tensor.matmul(
                                    st[:, hh * QC:(hh + 1) * QC],
                                    kt[p0:p0 + HD, pair, k * 128:(k + 1) * 128],
                                    qt[p0:p0 + HD, pair, q0:q0 + QC],
                                    start=True, stop=True,
                                    tile_position=(p0, 0))
                        w = wp.tile([128, 1024], DT, tag="w", name="w")
                        nc.scalar.activation(w, st, AF.Exp, scale=SCALE)
                        wtiles[k] = w
                        if k == 0 and carry is not None:
                            carry()
                            carry = None
                        if pinned:
                            for u in pinned.get(k, []):
                                u()
                        while fi < (k + 1) * len(fillers) // NB:
                            fillers[fi]()
                            fi += 1
                        if k > 0:
                            wprev = wtiles.pop(k - 1)
                            with tc.high_priority(offset=pv_prio):
                                for hh in range(2):
                                    nc.tensor.matmul(
                                        pvs[hh][:, :],
                                        vp[:, k - 1, 2 * pair + hh, 0:HD + 1],
                                        wprev[:, hh * QC:(hh + 1) * QC],
                                        start=(k - 1 == 0), stop=False)
                    wlast = wtiles.pop(NB - 1)

                    def finish():
                        for hh in range(2):
                            nc.tensor.matmul(
                                pvs[hh][:, :],
                                vp[:, NB - 1, 2 * pair + hh, 0:HD + 1],
                                wlast[:, hh * QC:(hh + 1) * QC],
                                start=False, stop=True)
                        # normalize both heads, ops interleaved so the GP
                        # broadcasts overlap the DVE chain; the at-mul reads
                        # pv straight from PSUM.
                        den = [dn.tile([1, QC], F32, tag=f"den{h}",
                                       name=f"den{h}") for h in range(2)]
                        rec = [dn.tile([1, QC], F32, tag=f"rec{h}",
                                       name=f"rec{h}") for h in range(2)]
                        bcr = [up.tile([HD, QC], F32, tag=f"bcr{h}",
                                       name=f"bcr{h}") for h in range(2)]
                        for hh in range(2):
                            nc.vector.tensor_copy(den[hh],
                                                  pvs[hh][HD:HD + 1, :])
                        for hh in range(2):
                            nc.vector.reciprocal_approx_fast(rec[hh],
                                                             den[hh])
                            nc.gpsimd.partition_broadcast(bcr[hh],
                                                          rec[hh][0:1, :])
                        for hh in range(2):
                            p0 = hh * HD
                            nc.vector.tensor_mul(
                                at[p0:p0 + HD, pair, q0:q0 + QC],
                                pvs[hh][0:HD, :], bcr[hh])
                    return finish

                # ---- filler inventory ----
                V = [vnat_group(i) for i in range(NB)]
                K0 = [proj_group(wkp[0], kt, bks, 0, g) for g in range(4)]
                K1 = [proj_group(wkp[1], kt, bks, 1, g) for g in range(4)]
                Q0 = [proj_group(wqp[0], qt, bqs, 0, g) for g in range(4)]
                Q1 = [proj_group(wqp[1], qt, bqs, 1, g) for g in range(4)]
                # tail units (qc=3) run after attention: they can rotate
                # over the freed attention PSUM banks and use ScalarE for
                # half the evictions.
                TT = [("pjA", "pjB"), ("pvA", "pvB"), ("s0", "s1")]
                tailq = [nc.sync, nc.gpsimd, nc.scalar, nc.sync]
                O = [[oproj_unit(4 * qc + nb,
                                 evict=("dve" if qc < 3 else
                                        ("dve", "act")[nb % 2]),
                                 tags=(None if qc < 3 else TT[nb % 3]),
                                 ring=(None if qc < 3 else tailq[nb]))
                      for nb in range(4)] for qc in range(NQC)]

                # Block (0,0): V[j] pinned at iter j+1 feeds PV(j) just in
                # time; K0[tg] pinned ahead of scores iter 4*tg, paced to
                # x token-group arrival.  Block (1,0): K1[g] ahead of its
                # scores iter 4*g.
                P00 = {1: [V[0]], 2: [V[1], K0[1]], 3: [V[2]], 4: [V[3]],
                       5: [V[4]], 6: [V[5], K0[2]], 7: [V[6]], 8: [V[7]],
                       9: [V[8], K0[3]], 10: [V[9]], 11: [V[10]],
                       12: [V[11], K1[0]], 13: [V[12]], 14: [V[13], Q1[0]],
                       15: [V[14], V[15]]}
                P10 = {0: [K1[1]], 3: [K1[2]], 7: [K1[3]], 10: [Q0[1]]}
                sched = [
                    (0, 0, [], P00),
                    (1, 0, [], P10),
                    (0, 1, [Q1[1]] + O[0][0:2], None),
                    (1, 1, O[0][2:4] + [Q0[2]], None),
                    (0, 2, [Q1[2]] + O[1][0:2], None),
                    (1, 2, O[1][2:4] + [Q0[3]], None),
                    (0, 3, [Q1[3]] + O[2][0:2], None),
                    (1, 3, O[2][2:4], None),
                ]
                fin = None
                for bi, (pair, qc, fillers, pinned) in enumerate(sched):
                    fin = attn_block(pair, qc, fillers, pinned=pinned,
                                     carry=fin,
                                     pv_prio=(1 << 19) if bi == len(sched) - 1
                                     else 0)
                fin()
                for g in O[3]:
                    g()
    return nc


_CACHE = {}


def _build():
    if "nc" not in _CACHE:
        nc = bacc.Bacc("TRN2", target_bir_lowering=False, debug=False)
        _emit(nc)
        nc.compile()
        _CACHE["nc"] = nc
    return _CACHE["nc"]


def _swiz(a):
    # [C*128, M] -> [128, C*M]: partition p gets row p of every 128-row chunk
    cm, m = a.shape
    c = cm // 128
    return np.ascontiguousarray(
        a.reshape(c, 128, m).transpose(1, 0, 2)).reshape(128, c * m)


def make_in_maps(x, Wq, bq, Wk, bk, Wv, bv, Wo, bo):
    import ml_dtypes
    f32 = np.float32
    bt = ml_dtypes.bfloat16
    ones_np = np.ones((128, NB, 4), bt)

    def tok_major(xb):
        # [128, (dc, n)] -> [128, (tg, dc, 512)]
        s = _swiz(np.ascontiguousarray(np.asarray(xb, f32).T).astype(bt))
        return np.ascontiguousarray(
            s.reshape(128, DC, TG, 512).transpose(0, 2, 1, 3)
        ).reshape(128, DC * N)

    def pair_major(w):
        # [128, (dc, 256)] -> [128, (pair, dc, 128)]
        return np.ascontiguousarray(
            w.reshape(128, DC, 2, 128).transpose(0, 2, 1, 3)
        ).reshape(128, DC * E)

    xTs = [tok_major(x[b]) for b in range(B)]
    in_maps = []
    for c in range(8):
        b, r0 = c // 4, (c % 4) * E
        rows = slice(r0, r0 + E)
        in_maps.append({
            "xT": xTs[b],
            "wqT": pair_major(_swiz(np.ascontiguousarray(
                np.asarray(Wq, f32)[rows].T).astype(bt))),
            "wkT": pair_major(_swiz(np.ascontiguousarray(
                np.asarray(Wk, f32)[rows].T).astype(bt))),
            "wvT": _swiz(np.ascontiguousarray(np.asarray(Wv, f32)[rows].T).astype(bt)),
            "woT": _swiz(np.ascontiguousarray(np.asarray(Wo, f32)[:, rows].T).astype(bt)),
            "bq2": np.ascontiguousarray(np.asarray(bq, f32)[rows].reshape(2, 128).T),
            "bk2": np.ascontiguousarray(np.asarray(bk, f32)[rows].reshape(2, 128).T),
            "bv1": np.ascontiguousarray(np.asarray(bv, f32)[rows]),
            "vones": ones_np,
        })
    return in_maps


def kernel(x, Wq, bq, Wk, bk, Wv, bv, Wo, bo, _spmd_kwargs=None):
    nc = _build()
    in_maps = make_in_maps(x, Wq, bq, Wk, bk, Wv, bv, Wo, bo)
    res = run_bass_kernel_spmd(nc, in_maps, core_ids=list(range(8)),
                               **(_spmd_kwargs or {}))
    parts = np.stack([np.asarray(res.results[c]["out"], np.float32)
                      for c in range(8)])
    outv = parts.reshape(B, 4, N, D).sum(axis=1) + np.asarray(bo, np.float32)
    if _spmd_kwargs:
        _CACHE["last_results"] = res
    return outv.astype(np.float32)


# revision 26
# speedup vs baseline: 1.3835x; 1.0254x over previous
"""Multi-head attention (B=2, N=2048, D=1024, H=16) on 8 Trainium2 cores.

Sharding: data-parallel over batch (cores 0-3 -> b=0, cores 4-7 -> b=1) and
tensor-parallel over heads (4 heads per core = 256 of 1024 QKV/O channels).
Each core computes its 4 heads' attention plus a partial output projection;
the host sums the 4 partials per batch and adds bo.

v4 pipeline (per core): the kernel is exp-chain bound (16.8M softmax exps
must stream through ScalarE's 128 lanes at ~1.15us per key-block), so the
win is starting that chain early and never stalling it.
 - x arrives TOKEN-MAJOR: the host lays xT out as (token-group, d-chunk)
   so K/Q projections for tokens 0-511 complete after ~1.5MB of DMA and
   the first scores+exp fire ~15us in, ~14us earlier than a d-chunk-major
   stream.  Remaining x token-groups stream in behind the running
   attention; the K projections for later token-groups and all V/Q/K-pair1
   projections run as PE filler units pinned to specific k-iterations of
   the first two attention blocks, paced to match their DMA arrival.
 - Attention processes a (pair, 512-query-chunk) block at a time.  Per
   k-iter the TWO heads of the pair run their scores matmuls CONCURRENTLY
   in disjoint PE row-groups (K=64 each, tile_position (0,0)/(64,0)) into
   the two halves of one [128,1024] PSUM tile; a single FD=1024 exp on
   ScalarE covers both heads; PV (M=65 with the ones/denominator column)
   runs per head with a one-iter lag.
 - ScalarE's DMA ring is used only during the intro (it is exp-bound
   after); output stores ride sync/gpsimd, plus scalar for the tail.
 - Normalization: den copy, reciprocal_approx_fast, GPSIMD
   partition_broadcast, one tensor_mul reading pv straight from PSUM.
"""

import numpy as np

import concourse.bass as bass
import concourse.bacc as bacc
import concourse.tile as tile
from concourse import mybir
from concourse.bass_utils import run_bass_kernel_spmd

F32 = mybir.dt.float32
BF16 = mybir.dt.bfloat16
AF = mybir.ActivationFunctionType

B, N, D, H, HD = 2, 2048, 1024, 16, 64
E = 256            # channels per core (4 heads * 64)
DC = D // 128      # 8 contraction chunks for projections
NB = N // 128      # 16 token blocks / k chunks
TG = 4             # 512-token groups (x DMA + K/Q projection granularity)
QC = 512           # query chunk
NQC = N // QC      # 4 query chunks
SCALE = 1.0 / np.sqrt(HD)
DT = BF16


def _emit(nc):
    # all big inputs arrive host-swizzled: per-partition contiguous rows so
    # the DMA descriptors are large (partition p owns columns p of every
    # 128-row chunk).  xT is token-group-major: [128, (tg, dc, 512)].
    # wqT/wkT are pair-major: [128, (pair, dc, 128)].
    xT = nc.dram_tensor("xT", [128, DC * N], DT, kind="ExternalInput")
    wqT = nc.dram_tensor("wqT", [128, DC * E], DT, kind="ExternalInput")
    wkT = nc.dram_tensor("wkT", [128, DC * E], DT, kind="ExternalInput")
    wvT = nc.dram_tensor("wvT", [128, DC * E], DT, kind="ExternalInput")
    woT = nc.dram_tensor("woT", [128, 2 * D], DT, kind="ExternalInput")
    # biases ride as FEW-partition rows: a [128, small] DMA costs one
    # descriptor PER PARTITION (~32ns each = ~4us of ring time), which
    # poisons the ring ahead of the x transfers.  [4, 128] is 4 packets.
    bqk4 = nc.dram_tensor("bqk4", [4, 132], F32, kind="ExternalInput")
    bv1 = nc.dram_tensor("bv1", [E], F32, kind="ExternalInput")
    out = nc.dram_tensor("out", [N, D], DT, kind="ExternalOutput")

    with tile.TileContext(nc) as tc:
        with tc.tile_pool(name="per", bufs=1) as per, \
             tc.tile_pool(name="wp", bufs=12) as wp, \
             tc.tile_pool(name="dn", bufs=2) as dn, \
             tc.tile_pool(name="up", bufs=2) as up, \
             tc.tile_pool(name="op", bufs=6) as op:

            # ---- persistent SBUF tiles ----
            # x arrives as (token-group, d-chunk-piece) tiles: pieces A/B/C
            # hold d-chunks 0-2 / 3-5 / 6-7 of one 512-token group, each a
            # single contiguous DMA.  Separate tiles per DMA so no false
            # write-order deps serialize them.
            xgA = [per.tile([128, 3, 512], DT, name=f"xgA{t}") for t in range(TG)]
            xgB = [per.tile([128, 3, 512], DT, name=f"xgB{t}") for t in range(TG)]
            xgC = [per.tile([128, 2, 512], DT, name=f"xgC{t}") for t in range(TG)]

            def xap(dc, lo, hi):
                tg, o, oh = lo // 512, lo % 512, hi - (lo // 512) * 512
                assert oh <= 512, "x slice crosses token groups"
                if dc < 3:
                    return xgA[tg][:, dc, o:oh]
                if dc < 6:
                    return xgB[tg][:, dc - 3, o:oh]
                return xgC[tg][:, dc - 6, o:oh]

            wqp = [per.tile([128, DC, 128], DT, name=f"wqp{i}") for i in range(2)]
            wkp = [per.tile([128, DC, 128], DT, name=f"wkp{i}") for i in range(2)]
            wv2 = [per.tile([128, 4, E], DT, name=f"wv2{i}") for i in range(2)]
            wv = [wv2[dc // 4][:, dc % 4, :] for dc in range(DC)]
            wo = per.tile([128, 2, D], DT)            # WoT (e-chunk)
            qt = per.tile([128, 2, N], DT)            # Q^T: (pair, tokens)
            kt = per.tile([128, 2, N], DT)
            vp = per.tile([128, NB, 4, 128], DT)      # V natural + ones col
            at = per.tile([128, 2, N], DT)            # attn^T normalized
            bsrc = per.tile([4, 132], F32)            # bias rows + 4x4 identity
            bqt = per.tile([128, 4], F32)             # [bq p0|p1, bk p0|p1]
            bvrow = per.tile([1, E], F32)
            bvb = per.tile([128, E], F32)
            ones64 = per.tile([128, 64], DT)

            qs = [nc.sync, nc.gpsimd]
            # ---- input DMA: need-ordered transfers on the 3 rings ----
            # Deadlines (at ~330GB/s aggregate): wk/wq pair0 + x tg0 by
            # ~13us (attention start), wv by ~15, x tg1 by ~18, x tg2 by
            # ~21, wk pair1 by ~23, x tg3 by ~25, wq pair1 / wo later.
            def xpiece(tg, lo, hi):
                return xT[:, tg * DC * 512 + lo * 512:
                          tg * DC * 512 + hi * 512].rearrange(
                    "p (c n) -> p c n", n=512)

            def wpair(dram, p):
                return dram[:, p * DC * 128:(p + 1) * DC * 128].rearrange(
                    "p (c e) -> p c e", e=128)

            # sync ring
            nc.sync.dma_start(out=wkp[0], in_=wpair(wkT, 0))
            nc.sync.dma_start(out=xgA[0], in_=xpiece(0, 0, 3))
            nc.sync.dma_start(out=xgA[1], in_=xpiece(1, 0, 3))
            nc.sync.dma_start(out=xgA[2], in_=xpiece(2, 0, 3))
            nc.sync.dma_start(out=xgA[3], in_=xpiece(3, 0, 3))
            # scalar ring (free until the first exp ~15us in)
            nc.scalar.dma_start(out=wqp[0], in_=wpair(wqT, 0))
            nc.scalar.dma_start(out=xgB[0], in_=xpiece(0, 3, 6))
            nc.scalar.dma_start(out=xgB[1], in_=xpiece(1, 3, 6))
            nc.scalar.dma_start(out=xgB[2], in_=xpiece(2, 3, 6))
            nc.scalar.dma_start(out=xgB[3], in_=xpiece(3, 3, 6))
            nc.scalar.dma_start(out=wqp[1], in_=wpair(wqT, 1))
            # gpsimd ring
            nc.gpsimd.dma_start(out=bsrc, in_=bqk4[:, :])
            nc.gpsimd.dma_start(out=bvrow, in_=bv1[:])
            nc.gpsimd.dma_start(out=xgC[0], in_=xpiece(0, 6, 8))
            nc.gpsimd.dma_start(out=wv2[0], in_=wvT[:, 0:4 * E].rearrange(
                "p (c e) -> p c e", e=E))
            nc.gpsimd.dma_start(out=wv2[1], in_=wvT[:, 4 * E:8 * E].rearrange(
                "p (c e) -> p c e", e=E))
            nc.gpsimd.dma_start(out=xgC[1], in_=xpiece(1, 6, 8))
            nc.gpsimd.dma_start(out=xgC[2], in_=xpiece(2, 6, 8))
            nc.gpsimd.dma_start(out=wkp[1], in_=wpair(wkT, 1))
            nc.gpsimd.dma_start(out=xgC[3], in_=xpiece(3, 6, 8))
            nc.gpsimd.dma_start(out=wo, in_=woT.rearrange(
                "p (c e) -> p c e", e=D))

            # ---- on-chip constants: ones column, bias spread ----
            nc.gpsimd.memset(ones64, 1.0)
            nc.vector.tensor_copy(vp[:, :, :, HD:HD + 1].rearrange(
                "p a b o -> p (a b o)"), ones64)
            nc.gpsimd.partition_broadcast(bvb, bvrow[0:1, :])

            # ---- pre-phase: bias transpose + K pair0 tg0 + Q pair0 g0 ----
            # (all other projections run as pinned/paced fillers inside the
            # attention blocks, matched to DMA arrival order)
            with tc.tile_pool(name="pre", bufs=1, space="PSUM") as pre:
                pb = pre.tile([128, 4], F32, tag="p2", name="pb")
                nc.tensor.matmul(pb, bsrc[:, 0:128], bsrc[:, 128:132],
                                 start=True, stop=True)
                nc.vector.tensor_copy(bqt, pb)
                bqs = bqt[:, 0:2]
                bks = bqt[:, 2:4]
                pk0 = pre.tile([128, 512], F32, tag="p0", name="pk0")
                pq0 = pre.tile([128, 512], F32, tag="p1", name="pq0")
                for dc in range(DC):
                    nc.tensor.matmul(
                        pk0, wkp[0][:, dc, :], xap(dc, 0, 512),
                        start=(dc == 0), stop=(dc == DC - 1))
                for dc in range(DC):
                    nc.tensor.matmul(
                        pq0, wqp[0][:, dc, :], xap(dc, 0, 512),
                        start=(dc == 0), stop=(dc == DC - 1))
                with tc.high_priority(offset=1 << 19):
                    nc.vector.tensor_scalar_add(kt[:, 0, 0:512], pk0,
                                                bks[:, 0:1])
                    nc.vector.tensor_scalar_add(qt[:, 0, 0:512], pq0,
                                                bqs[:, 0:1])

            with tc.tile_pool(name="ps", bufs=1, space="PSUM") as ps:
                pj_n = [0]

                def pj_tag():
                    pj_n[0] += 1
                    return ("pjA", "pjB")[pj_n[0] % 2]

                # ---- filler units (1-bank psum groups on pj tags) ----
                # K/Q projection groups run at boosted priority: they feed
                # upcoming scores and must not queue behind the
                # V-projection / PV backlog.
                def proj_group(wpt, dst, bias, pair, n4):
                    def emit():
                        with tc.high_priority(offset=1 << 19):
                            pt = ps.tile([128, 512], F32, tag=pj_tag(),
                                         name="ppj")
                            for dc in range(DC):
                                nc.tensor.matmul(
                                    pt[:, :],
                                    wpt[:, dc, :],
                                    xap(dc, n4 * 512, (n4 + 1) * 512),
                                    start=(dc == 0), stop=(dc == DC - 1))
                            nc.vector.tensor_scalar_add(
                                dst[:, pair, n4 * 512:(n4 + 1) * 512], pt[:, :],
                                bias[:, pair:pair + 1])
                    return emit

                def vnat_group(nb):
                    def emit():
                        pt = ps.tile([128, E], F32, tag=pj_tag(), name="pvn")
                        for dc in range(DC):
                            nc.tensor.matmul(
                                pt[:, :],
                                xap(dc, nb * 128, (nb + 1) * 128),
                                wv[dc],
                                start=(dc == 0), stop=(dc == DC - 1))
                        nc.vector.tensor_add(
                            vp[:, nb, :, 0:HD],
                            pt.rearrange("p (h d) -> p h d", h=4),
                            bvb.rearrange("p (h d) -> p h d", h=4))
                    return emit

                o_n = [0]

                def oproj_unit(nb, evict="dve", tags=None, ring=None):
                    # both D-halves of a 128-token block -> one contiguous
                    # [128, 1024] row store
                    def emit():
                        ot = op.tile([128, 1024], DT, tag="ot", name="ot")
                        pos = [ps.tile([128, 512], F32,
                                       tag=(tags[h] if tags else pj_tag()),
                                       name="po")
                               for h in range(2)]
                        # ec-outer: each at-chunk stationary serves both
                        # D-halves consecutively
                        for ec in range(2):
                            for half in range(2):
                                nc.tensor.matmul(
                                    pos[half][:, :],
                                    at[:, ec, nb * 128:(nb + 1) * 128],
                                    wo[:, ec, half * 512:(half + 1) * 512],
                                    start=(ec == 0), stop=(ec == 1))
                        for half in range(2):
                            if evict == "dve":
                                nc.vector.tensor_copy(
                                    ot[:, half * 512:(half + 1) * 512],
                                    pos[half])
                            else:
                                nc.scalar.copy(
                                    ot[:, half * 512:(half + 1) * 512],
                                    pos[half])
                        o_n[0] += 1
                        (ring or qs[o_n[0] % 2]).dma_start(
                            out=out[nb * 128:(nb + 1) * 128, :], in_=ot)
                    return emit

                # ---- one (pair, qc) attention block: 16 k-iters ----
                # Returns a `finish` closure (last PV pair + normalization);
                # the caller runs it after the NEXT block's first k-iter so
                # ScalarE never stalls across block boundaries.  `pinned`
                # maps k-iter -> filler units that must run at that iter
                # (used to match DMA arrival order in the first blocks).
                def attn_block(pair, qc, fillers, pinned=None, carry=None,
                               pv_prio=0):
                    q0 = qc * QC
                    fi = 0
                    pvs = [ps.tile([HD + 1, QC], F32, tag=t, name=t)
                           for t in ("pvA", "pvB")]
                    wtiles = {}
                    for k in range(NB):
                        st = ps.tile([128, 1024], F32,
                                     tag=("s0", "s1")[k % 2], name="st")
                        # scores for both heads back-to-back at max priority
                        # so they sit adjacent in the PE queue and overlap in
                        # disjoint row-groups of the array.
                        with tc.high_priority(offset=1 << 20):
                            for hh in range(2):
                                p0 = hh * HD
                                nc.tensor.matmul(
                                    st[:, hh * QC:(hh + 1) * QC],
                                    kt[p0:p0 + HD, pair, k * 128:(k + 1) * 128],
                                    qt[p0:p0 + HD, pair, q0:q0 + QC],
                                    start=True, stop=True,
                                    tile_position=(p0, 0))
                        w = wp.tile([128, 1024], DT, tag="w", name="w")
                        nc.scalar.activation(w, st, AF.Exp, scale=SCALE)
                        wtiles[k] = w
                        if k == 0 and carry is not None:
                            carry()
                            carry = None
                        if pinned:
                            for u in pinned.get(k, []):
                                u()
                        while fi < (k + 1) * len(fillers) // NB:
                            fillers[fi]()
                            fi += 1
                        if k > 0:
                            wprev = wtiles.pop(k - 1)
                            with tc.high_priority(offset=pv_prio):
                                for hh in range(2):
                                    nc.tensor.matmul(
                                        pvs[hh][:, :],
                                        vp[:, k - 1, 2 * pair + hh, 0:HD + 1],
                                        wprev[:, hh * QC:(hh + 1) * QC],
                                        start=(k - 1 == 0), stop=False)
                    wlast = wtiles.pop(NB - 1)

                    def finish():
                        for hh in range(2):
                            nc.tensor.matmul(
                                pvs[hh][:, :],
                                vp[:, NB - 1, 2 * pair + hh, 0:HD + 1],
                                wlast[:, hh * QC:(hh + 1) * QC],
                                start=False, stop=True)
                        # normalize both heads, ops interleaved so the GP
                        # broadcasts overlap the DVE chain; the at-mul reads
                        # pv straight from PSUM.
                        den = [dn.tile([1, QC], F32, tag=f"den{h}",
                                       name=f"den{h}") for h in range(2)]
                        rec = [dn.tile([1, QC], F32, tag=f"rec{h}",
                                       name=f"rec{h}") for h in range(2)]
                        bcr = [up.tile([HD, QC], F32, tag=f"bcr{h}",
                                       name=f"bcr{h}") for h in range(2)]
                        for hh in range(2):
                            nc.vector.tensor_copy(den[hh],
                                                  pvs[hh][HD:HD + 1, :])
                        for hh in range(2):
                            nc.vector.reciprocal_approx_fast(rec[hh],
                                                             den[hh])
                            nc.gpsimd.partition_broadcast(bcr[hh],
                                                          rec[hh][0:1, :])
                        for hh in range(2):
                            p0 = hh * HD
                            nc.vector.tensor_mul(
                                at[p0:p0 + HD, pair, q0:q0 + QC],
                                pvs[hh][0:HD, :], bcr[hh])
                    return finish

                # ---- filler inventory ----
                V = [vnat_group(i) for i in range(NB)]
                K0 = [proj_group(wkp[0], kt, bks, 0, g) for g in range(4)]
                K1 = [proj_group(wkp[1], kt, bks, 1, g) for g in range(4)]
                Q0 = [proj_group(wqp[0], qt, bqs, 0, g) for g in range(4)]
                Q1 = [proj_group(wqp[1], qt, bqs, 1, g) for g in range(4)]
                # tail units (qc=3) run after attention: they can rotate
                # over the freed attention PSUM banks and use ScalarE for
                # half the evictions.
                TT = [("pjA", "pjB"), ("pvA", "pvB"), ("s0", "s1")]
                tailq = [nc.sync, nc.gpsimd, nc.scalar, nc.sync]
                O = [[oproj_unit(4 * qc + nb,
                                 evict=("dve" if qc < 3 else
                                        ("dve", "act")[nb % 2]),
                                 tags=(None if qc < 3 else TT[nb % 3]),
                                 ring=(None if qc < 3 else tailq[nb]))
                      for nb in range(4)] for qc in range(NQC)]

                # Block (0,0): V[j] pinned at iter j+1 feeds PV(j) just in
                # time; K0[tg] pinned ahead of scores iter 4*tg, paced to
                # x token-group arrival.  Block (1,0): K1[g] ahead of its
                # scores iter 4*g.
                P00 = {1: [V[0]], 2: [V[1], K0[1]], 3: [V[2]], 4: [V[3]],
                       5: [V[4]], 6: [V[5], K0[2]], 7: [V[6]], 8: [V[7]],
                       9: [V[8], K0[3]], 10: [V[9]], 11: [V[10]],
                       12: [V[11], K1[0]], 13: [V[12]], 14: [V[13], Q1[0]],
                       15: [V[14], V[15]]}
                P10 = {0: [K1[1]], 3: [K1[2]], 7: [K1[3]], 10: [Q0[1]]}
                sched = [
                    (0, 0, [], P00),
                    (1, 0, [], P10),
                    (0, 1, [Q1[1]] + O[0][0:2], None),
                    (1, 1, O[0][2:4] + [Q0[2]], None),
                    (0, 2, [Q1[2]] + O[1][0:2], None),
                    (1, 2, O[1][2:4] + [Q0[3]], None),
                    (0, 3, [Q1[3]] + O[2][0:2], None),
                    (1, 3, O[2][2:4], None),
                ]
                fin = None
                for bi, (pair, qc, fillers, pinned) in enumerate(sched):
                    # last block: PV above even the scores priority so the
                    # PV backlog drains with the exp chain instead of after
                    # it (it gates the tail's norm + final oproj).
                    fin = attn_block(pair, qc, fillers, pinned=pinned,
                                     carry=fin,
                                     pv_prio=(1 << 21) if bi == len(sched) - 1
                                     else 0)
                fin()
                for g in O[3]:
                    g()
    return nc


_CACHE = {}


def _build():
    if "nc" not in _CACHE:
        nc = bacc.Bacc("TRN2", target_bir_lowering=False, debug=False)
        _emit(nc)
        nc.compile()
        _CACHE["nc"] = nc
    return _CACHE["nc"]


def _swiz(a):
    # [C*128, M] -> [128, C*M]: partition p gets row p of every 128-row chunk
    cm, m = a.shape
    c = cm // 128
    return np.ascontiguousarray(
        a.reshape(c, 128, m).transpose(1, 0, 2)).reshape(128, c * m)


def make_in_maps(x, Wq, bq, Wk, bk, Wv, bv, Wo, bo):
    import ml_dtypes
    f32 = np.float32
    bt = ml_dtypes.bfloat16

    def tok_major(xb):
        # [128, (dc, n)] -> [128, (tg, dc, 512)]
        s = _swiz(np.ascontiguousarray(np.asarray(xb, f32).T).astype(bt))
        return np.ascontiguousarray(
            s.reshape(128, DC, TG, 512).transpose(0, 2, 1, 3)
        ).reshape(128, DC * N)

    def pair_major(w):
        # [128, (dc, 256)] -> [128, (pair, dc, 128)]
        return np.ascontiguousarray(
            w.reshape(128, DC, 2, 128).transpose(0, 2, 1, 3)
        ).reshape(128, DC * E)

    xTs = [tok_major(x[b]) for b in range(B)]
    in_maps = []
    for c in range(8):
        b, r0 = c // 4, (c % 4) * E
        rows = slice(r0, r0 + E)
        in_maps.append({
            "xT": xTs[b],
            "wqT": pair_major(_swiz(np.ascontiguousarray(
                np.asarray(Wq, f32)[rows].T).astype(bt))),
            "wkT": pair_major(_swiz(np.ascontiguousarray(
                np.asarray(Wk, f32)[rows].T).astype(bt))),
            "wvT": _swiz(np.ascontiguousarray(np.asarray(Wv, f32)[rows].T).astype(bt)),
            "woT": _swiz(np.ascontiguousarray(np.asarray(Wo, f32)[:, rows].T).astype(bt)),
            "bqk4": np.ascontiguousarray(np.concatenate(
                [np.concatenate([np.asarray(bq, f32)[rows].reshape(2, 128),
                                 np.asarray(bk, f32)[rows].reshape(2, 128)],
                                axis=0),
                 np.eye(4, dtype=f32)], axis=1)),
            "bv1": np.ascontiguousarray(np.asarray(bv, f32)[rows]),
        })
    return in_maps


def kernel(x, Wq, bq, Wk, bk, Wv, bv, Wo, bo, _spmd_kwargs=None):
    nc = _build()
    in_maps = make_in_maps(x, Wq, bq, Wk, bk, Wv, bv, Wo, bo)
    res = run_bass_kernel_spmd(nc, in_maps, core_ids=list(range(8)),
                               **(_spmd_kwargs or {}))
    parts = np.stack([np.asarray(res.results[c]["out"], np.float32)
                      for c in range(8)])
    outv = parts.reshape(B, 4, N, D).sum(axis=1) + np.asarray(bo, np.float32)
    if _spmd_kwargs:
        _CACHE["last_results"] = res
    return outv.astype(np.float32)


# revision 30
# speedup vs baseline: 1.3949x; 1.0082x over previous
"""Multi-head attention (B=2, N=2048, D=1024, H=16) on 8 Trainium2 cores.

Sharding: data-parallel over batch (cores 0-3 -> b=0, cores 4-7 -> b=1) and
tensor-parallel over heads (4 heads per core = 256 of 1024 QKV/O channels).
Each core computes its 4 heads' attention plus a partial output projection;
the host sums the 4 partials per batch and adds bo.

v4 pipeline (per core): the kernel is exp-chain bound (16.8M softmax exps
must stream through ScalarE's 128 lanes at ~1.15us per key-block), so the
win is starting that chain early and never stalling it.
 - x arrives TOKEN-MAJOR: the host lays xT out as (token-group, d-chunk)
   so K/Q projections for tokens 0-511 complete after ~1.5MB of DMA and
   the first scores+exp fire ~15us in, ~14us earlier than a d-chunk-major
   stream.  Remaining x token-groups stream in behind the running
   attention; the K projections for later token-groups and all V/Q/K-pair1
   projections run as PE filler units pinned to specific k-iterations of
   the first two attention blocks, paced to match their DMA arrival.
 - Attention processes a (pair, 512-query-chunk) block at a time.  Per
   k-iter the TWO heads of the pair run their scores matmuls CONCURRENTLY
   in disjoint PE row-groups (K=64 each, tile_position (0,0)/(64,0)) into
   the two halves of one [128,1024] PSUM tile; a single FD=1024 exp on
   ScalarE covers both heads; PV (M=65 with the ones/denominator column)
   runs per head with a one-iter lag.
 - ScalarE's DMA ring is used only during the intro (it is exp-bound
   after); output stores ride sync/gpsimd, plus scalar for the tail.
 - Normalization: den copy, reciprocal_approx_fast, GPSIMD
   partition_broadcast, one tensor_mul reading pv straight from PSUM.
"""

import numpy as np

import concourse.bass as bass
import concourse.bacc as bacc
import concourse.tile as tile
from concourse import mybir
from concourse.bass_utils import run_bass_kernel_spmd

F32 = mybir.dt.float32
BF16 = mybir.dt.bfloat16
AF = mybir.ActivationFunctionType

B, N, D, H, HD = 2, 2048, 1024, 16, 64
E = 256            # channels per core (4 heads * 64)
DC = D // 128      # 8 contraction chunks for projections
NB = N // 128      # 16 token blocks / k chunks
TG = 4             # 512-token groups (x DMA + K/Q projection granularity)
QC = 512           # query chunk
NQC = N // QC      # 4 query chunks
SCALE = 1.0 / np.sqrt(HD)
DT = BF16


def _emit(nc):
    # all big inputs arrive host-swizzled: per-partition contiguous rows so
    # the DMA descriptors are large (partition p owns columns p of every
    # 128-row chunk).  xT is token-group-major: [128, (tg, dc, 512)].
    # wqT/wkT are pair-major: [128, (pair, dc, 128)].
    xT = nc.dram_tensor("xT", [128, DC * N], DT, kind="ExternalInput")
    wqT = nc.dram_tensor("wqT", [128, DC * E], DT, kind="ExternalInput")
    wkT = nc.dram_tensor("wkT", [128, DC * E], DT, kind="ExternalInput")
    wvT = nc.dram_tensor("wvT", [128, DC * E], DT, kind="ExternalInput")
    woT = nc.dram_tensor("woT", [128, 2 * D], DT, kind="ExternalInput")
    # biases ride as FEW-partition rows: a [128, small] DMA costs one
    # descriptor PER PARTITION (~32ns each = ~4us of ring time), which
    # poisons the ring ahead of the x transfers.  [4, 128] is 4 packets.
    bqk4 = nc.dram_tensor("bqk4", [4, 132], F32, kind="ExternalInput")
    bv1 = nc.dram_tensor("bv1", [E], F32, kind="ExternalInput")
    out = nc.dram_tensor("out", [N, D], DT, kind="ExternalOutput")

    with tile.TileContext(nc) as tc:
        with tc.tile_pool(name="per", bufs=1) as per, \
             tc.tile_pool(name="wp", bufs=12) as wp, \
             tc.tile_pool(name="dn", bufs=2) as dn, \
             tc.tile_pool(name="up", bufs=2) as up, \
             tc.tile_pool(name="op", bufs=6) as op:

            # ---- persistent SBUF tiles ----
            # x arrives as (token-group, d-chunk-piece) tiles: pieces A/B/C
            # hold d-chunks 0-2 / 3-5 / 6-7 of one 512-token group, each a
            # single contiguous DMA.  Separate tiles per DMA so no false
            # write-order deps serialize them.
            xgA = [per.tile([128, 3, 512], DT, name=f"xgA{t}") for t in range(TG)]
            xgB = [per.tile([128, 3, 512], DT, name=f"xgB{t}") for t in range(TG)]
            xgC = [per.tile([128, 2, 512], DT, name=f"xgC{t}") for t in range(TG)]

            def xap(dc, lo, hi):
                tg, o, oh = lo // 512, lo % 512, hi - (lo // 512) * 512
                assert oh <= 512, "x slice crosses token groups"
                if dc < 3:
                    return xgA[tg][:, dc, o:oh]
                if dc < 6:
                    return xgB[tg][:, dc - 3, o:oh]
                return xgC[tg][:, dc - 6, o:oh]

            wqp = [per.tile([128, DC, 128], DT, name=f"wqp{i}") for i in range(2)]
            wkp = [per.tile([128, DC, 128], DT, name=f"wkp{i}") for i in range(2)]
            wv2 = [per.tile([128, 4, E], DT, name=f"wv2{i}") for i in range(2)]
            wv = [wv2[dc // 4][:, dc % 4, :] for dc in range(DC)]
            wo = per.tile([128, 2, D], DT)            # WoT (e-chunk)
            qt = per.tile([128, 2, N], DT)            # Q^T: (pair, tokens)
            kt = per.tile([128, 2, N], DT)
            vp = per.tile([128, NB, 4, 128], DT)      # V natural + ones col
            at = per.tile([128, 2, N], DT)            # attn^T normalized
            bsrc = per.tile([4, 132], F32)            # bias rows + 4x4 identity
            bqt = per.tile([128, 4], F32)             # [bq p0|p1, bk p0|p1]
            bvrow = per.tile([1, E], F32)
            bvb = per.tile([128, E], F32)
            ones64 = per.tile([128, 64], DT)

            qs = [nc.sync, nc.gpsimd]
            # ---- input DMA: need-ordered transfers on the 3 rings ----
            # Deadlines (at ~330GB/s aggregate): wk/wq pair0 + x tg0 by
            # ~13us (attention start), wv by ~15, x tg1 by ~18, x tg2 by
            # ~21, wk pair1 by ~23, x tg3 by ~25, wq pair1 / wo later.
            def xpiece(tg, lo, hi):
                return xT[:, tg * DC * 512 + lo * 512:
                          tg * DC * 512 + hi * 512].rearrange(
                    "p (c n) -> p c n", n=512)

            def wpair(dram, p):
                return dram[:, p * DC * 128:(p + 1) * DC * 128].rearrange(
                    "p (c e) -> p c e", e=128)

            # sync ring
            nc.sync.dma_start(out=wkp[0], in_=wpair(wkT, 0))
            nc.sync.dma_start(out=xgA[0], in_=xpiece(0, 0, 3))
            nc.sync.dma_start(out=xgA[1], in_=xpiece(1, 0, 3))
            nc.sync.dma_start(out=xgA[2], in_=xpiece(2, 0, 3))
            nc.sync.dma_start(out=xgA[3], in_=xpiece(3, 0, 3))
            # scalar ring (free until the first exp ~15us in)
            nc.scalar.dma_start(out=wqp[0], in_=wpair(wqT, 0))
            nc.scalar.dma_start(out=xgB[0], in_=xpiece(0, 3, 6))
            nc.scalar.dma_start(out=xgB[1], in_=xpiece(1, 3, 6))
            nc.scalar.dma_start(out=xgB[2], in_=xpiece(2, 3, 6))
            nc.scalar.dma_start(out=xgB[3], in_=xpiece(3, 3, 6))
            nc.scalar.dma_start(out=wqp[1], in_=wpair(wqT, 1))
            # gpsimd ring
            nc.gpsimd.dma_start(out=bsrc, in_=bqk4[:, :])
            nc.gpsimd.dma_start(out=bvrow, in_=bv1[:])
            nc.gpsimd.dma_start(out=xgC[0], in_=xpiece(0, 6, 8))
            nc.gpsimd.dma_start(out=wv2[0], in_=wvT[:, 0:4 * E].rearrange(
                "p (c e) -> p c e", e=E))
            nc.gpsimd.dma_start(out=wv2[1], in_=wvT[:, 4 * E:8 * E].rearrange(
                "p (c e) -> p c e", e=E))
            nc.gpsimd.dma_start(out=xgC[1], in_=xpiece(1, 6, 8))
            nc.gpsimd.dma_start(out=xgC[2], in_=xpiece(2, 6, 8))
            nc.gpsimd.dma_start(out=wkp[1], in_=wpair(wkT, 1))
            nc.gpsimd.dma_start(out=xgC[3], in_=xpiece(3, 6, 8))
            nc.gpsimd.dma_start(out=wo, in_=woT.rearrange(
                "p (c e) -> p c e", e=D))

            # ---- on-chip constants: ones column, bias spread ----
            nc.gpsimd.memset(ones64, 1.0)
            nc.vector.tensor_copy(vp[:, :, :, HD:HD + 1].rearrange(
                "p a b o -> p (a b o)"), ones64)
            nc.gpsimd.partition_broadcast(bvb, bvrow[0:1, :])

            # ---- pre-phase: bias transpose + K pair0 tg0 + Q pair0 g0 ----
            # (all other projections run as pinned/paced fillers inside the
            # attention blocks, matched to DMA arrival order)
            with tc.tile_pool(name="pre", bufs=1, space="PSUM") as pre:
                pb = pre.tile([128, 4], F32, tag="p2", name="pb")
                nc.tensor.matmul(pb, bsrc[:, 0:128], bsrc[:, 128:132],
                                 start=True, stop=True)
                nc.vector.tensor_copy(bqt, pb)
                bqs = bqt[:, 0:2]
                bks = bqt[:, 2:4]
                pk0 = pre.tile([128, 512], F32, tag="p0", name="pk0")
                pq0 = pre.tile([128, 512], F32, tag="p1", name="pq0")
                for dc in range(DC):
                    nc.tensor.matmul(
                        pk0, wkp[0][:, dc, :], xap(dc, 0, 512),
                        start=(dc == 0), stop=(dc == DC - 1))
                for dc in range(DC):
                    nc.tensor.matmul(
                        pq0, wqp[0][:, dc, :], xap(dc, 0, 512),
                        start=(dc == 0), stop=(dc == DC - 1))
                with tc.high_priority(offset=1 << 19):
                    nc.vector.tensor_scalar_add(kt[:, 0, 0:512], pk0,
                                                bks[:, 0:1])
                    nc.vector.tensor_scalar_add(qt[:, 0, 0:512], pq0,
                                                bqs[:, 0:1])

            with tc.tile_pool(name="ps", bufs=1, space="PSUM") as ps:
                pj_n = [0]

                def pj_tag():
                    pj_n[0] += 1
                    return ("pjA", "pjB")[pj_n[0] % 2]

                # ---- filler units (1-bank psum groups on pj tags) ----
                # K/Q projection groups run at boosted priority: they feed
                # upcoming scores and must not queue behind the
                # V-projection / PV backlog.
                def proj_group(wpt, dst, bias, pair, n4):
                    def emit():
                        with tc.high_priority(offset=1 << 19):
                            pt = ps.tile([128, 512], F32, tag=pj_tag(),
                                         name="ppj")
                            for dc in range(DC):
                                nc.tensor.matmul(
                                    pt[:, :],
                                    wpt[:, dc, :],
                                    xap(dc, n4 * 512, (n4 + 1) * 512),
                                    start=(dc == 0), stop=(dc == DC - 1))
                            nc.vector.tensor_scalar_add(
                                dst[:, pair, n4 * 512:(n4 + 1) * 512], pt[:, :],
                                bias[:, pair:pair + 1])
                    return emit

                def vnat_group(nb):
                    def emit():
                        pt = ps.tile([128, E], F32, tag=pj_tag(), name="pvn")
                        for dc in range(DC):
                            nc.tensor.matmul(
                                pt[:, :],
                                xap(dc, nb * 128, (nb + 1) * 128),
                                wv[dc],
                                start=(dc == 0), stop=(dc == DC - 1))
                        nc.vector.tensor_add(
                            vp[:, nb, :, 0:HD],
                            pt.rearrange("p (h d) -> p h d", h=4),
                            bvb.rearrange("p (h d) -> p h d", h=4))
                    return emit

                o_n = [0]

                def oproj_unit(nb, evict="dve", tags=None, ring=None):
                    # both D-halves of a 128-token block -> one contiguous
                    # [128, 1024] row store
                    def emit():
                        ot = op.tile([128, 1024], DT, tag="ot", name="ot")
                        pos = [ps.tile([128, 512], F32,
                                       tag=(tags[h] if tags else pj_tag()),
                                       name="po")
                               for h in range(2)]
                        # ec-outer: each at-chunk stationary serves both
                        # D-halves consecutively
                        for ec in range(2):
                            for half in range(2):
                                nc.tensor.matmul(
                                    pos[half][:, :],
                                    at[:, ec, nb * 128:(nb + 1) * 128],
                                    wo[:, ec, half * 512:(half + 1) * 512],
                                    start=(ec == 0), stop=(ec == 1))
                        for half in range(2):
                            if evict == "dve":
                                nc.vector.tensor_copy(
                                    ot[:, half * 512:(half + 1) * 512],
                                    pos[half])
                            else:
                                nc.scalar.copy(
                                    ot[:, half * 512:(half + 1) * 512],
                                    pos[half])
                        o_n[0] += 1
                        (ring or qs[o_n[0] % 2]).dma_start(
                            out=out[nb * 128:(nb + 1) * 128, :], in_=ot)
                    return emit

                # ---- one (pair, qc) attention block: 16 k-iters ----
                # Returns a `finish` closure (last PV pair + normalization);
                # the caller runs it after the NEXT block's first k-iter so
                # ScalarE never stalls across block boundaries.  `pinned`
                # maps k-iter -> filler units that must run at that iter
                # (used to match DMA arrival order in the first blocks).
                def attn_block(pair, qc, fillers, pinned=None, carry=None,
                               pv_prio=0, split=False):
                    q0 = qc * QC
                    fi = 0
                    pvs = [ps.tile([HD + 1, QC], F32, tag=t, name=t)
                           for t in ("pvA", "pvB")]
                    wtiles = {}
                    for k in range(NB):
                        st = ps.tile([128, 1024], F32,
                                     tag=("s0", "s1")[k % 2], name="st")
                        # scores for both heads back-to-back at max priority
                        # so they sit adjacent in the PE queue and overlap in
                        # disjoint row-groups of the array.
                        with tc.high_priority(offset=1 << 20):
                            for hh in range(2):
                                p0 = hh * HD
                                nc.tensor.matmul(
                                    st[:, hh * QC:(hh + 1) * QC],
                                    kt[p0:p0 + HD, pair, k * 128:(k + 1) * 128],
                                    qt[p0:p0 + HD, pair, q0:q0 + QC],
                                    start=True, stop=True,
                                    tile_position=(p0, 0))
                        w = wp.tile([128, 1024], DT, tag="w", name="w")
                        nc.scalar.activation(w, st, AF.Exp, scale=SCALE)
                        wtiles[k] = w
                        if k == 0 and carry is not None:
                            carry()
                            carry = None
                        if pinned:
                            for u in pinned.get(k, []):
                                u()
                        while fi < (k + 1) * len(fillers) // NB:
                            fillers[fi]()
                            fi += 1
                        if k > 0:
                            wprev = wtiles.pop(k - 1)
                            with tc.high_priority(offset=pv_prio):
                                for hh in range(2):
                                    nc.tensor.matmul(
                                        pvs[hh][:, :],
                                        vp[:, k - 1, 2 * pair + hh, 0:HD + 1],
                                        wprev[:, hh * QC:(hh + 1) * QC],
                                        start=(k - 1 == 0), stop=False)
                    wlast = wtiles.pop(NB - 1)

                    def finish_pv():
                        for hh in range(2):
                            nc.tensor.matmul(
                                pvs[hh][:, :],
                                vp[:, NB - 1, 2 * pair + hh, 0:HD + 1],
                                wlast[:, hh * QC:(hh + 1) * QC],
                                start=False, stop=True)

                    def finish_norm():
                        # normalize both heads, ops interleaved so the GP
                        # broadcasts overlap the DVE chain; the at-mul reads
                        # pv straight from PSUM.
                        den = [dn.tile([1, QC], F32, tag=f"den{h}",
                                       name=f"den{h}") for h in range(2)]
                        rec = [dn.tile([1, QC], F32, tag=f"rec{h}",
                                       name=f"rec{h}") for h in range(2)]
                        bcr = [up.tile([HD, QC], F32, tag=f"bcr{h}",
                                       name=f"bcr{h}") for h in range(2)]
                        for hh in range(2):
                            nc.vector.tensor_copy(den[hh],
                                                  pvs[hh][HD:HD + 1, :])
                        for hh in range(2):
                            nc.vector.reciprocal_approx_fast(rec[hh],
                                                             den[hh])
                            nc.gpsimd.partition_broadcast(bcr[hh],
                                                          rec[hh][0:1, :])
                        for hh in range(2):
                            p0 = hh * HD
                            nc.vector.tensor_mul(
                                at[p0:p0 + HD, pair, q0:q0 + QC],
                                pvs[hh][0:HD, :], bcr[hh])

                    def finish():
                        finish_pv()
                        finish_norm()
                    return (finish_pv, finish_norm) if split else finish

                # ---- filler inventory ----
                V = [vnat_group(i) for i in range(NB)]
                K0 = [proj_group(wkp[0], kt, bks, 0, g) for g in range(4)]
                K1 = [proj_group(wkp[1], kt, bks, 1, g) for g in range(4)]
                Q0 = [proj_group(wqp[0], qt, bqs, 0, g) for g in range(4)]
                Q1 = [proj_group(wqp[1], qt, bqs, 1, g) for g in range(4)]
                O = [[oproj_unit(4 * qc + nb) for nb in range(4)]
                     for qc in range(3)]

                # tail units (qc=3, after the last block): pair-0 oproj
                # halves can run DURING the final normalization (they only
                # need `at` pair 0, written two blocks earlier) — keeping
                # the PE warm so HAM doesn't re-throttle; pair-1 halves,
                # evictions (DVE + ScalarE in parallel) and per-half
                # stores follow the norm.
                def tail_unit(nb, tags, rings):
                    state = {}

                    def begin():
                        pos = [ps.tile([128, 512], F32, tag=tags[h],
                                       name="po") for h in range(2)]
                        state["pos"] = pos
                        for half in range(2):
                            nc.tensor.matmul(
                                pos[half][:, :],
                                at[:, 0, nb * 128:(nb + 1) * 128],
                                wo[:, 0, half * 512:(half + 1) * 512],
                                start=True, stop=False)

                    def end():
                        pos = state["pos"]
                        ot = op.tile([128, 1024], DT, tag="ot", name="ot")
                        for half in range(2):
                            nc.tensor.matmul(
                                pos[half][:, :],
                                at[:, 1, nb * 128:(nb + 1) * 128],
                                wo[:, 1, half * 512:(half + 1) * 512],
                                start=False, stop=True)
                        for half in range(2):
                            if half == 0:
                                nc.vector.tensor_copy(
                                    ot[:, 0:512], pos[0])
                            else:
                                nc.scalar.copy(
                                    ot[:, 512:1024], pos[1])
                            rings[half].dma_start(
                                out=out[nb * 128:(nb + 1) * 128,
                                        half * 512:(half + 1) * 512],
                                in_=ot[:, half * 512:(half + 1) * 512])
                    return begin, end

                tails = [tail_unit(12, ("pjA", "pjB"), (nc.sync, nc.gpsimd)),
                         tail_unit(13, ("s0", "s1"), (nc.scalar, nc.sync)),
                         tail_unit(14, ("pvA", "pvB"), (nc.gpsimd, nc.scalar)),
                         tail_unit(15, ("pjA", "pjB"), (nc.sync, nc.gpsimd))]

                # Block (0,0): V[j] pinned at iter j+1 feeds PV(j) just in
                # time; K0[tg] pinned ahead of scores iter 4*tg, paced to
                # x token-group arrival.  Block (1,0): K1[g] ahead of its
                # scores iter 4*g.
                P00 = {1: [V[0]], 2: [V[1], K0[1]], 3: [V[2]], 4: [V[3]],
                       5: [V[4]], 6: [V[5], K0[2]], 7: [V[6]], 8: [V[7]],
                       9: [V[8], K0[3]], 10: [V[9]], 11: [V[10]],
                       12: [V[11], K1[0]], 13: [V[12]], 14: [V[13], Q1[0]],
                       15: [V[14], V[15]]}
                P10 = {0: [K1[1]], 3: [K1[2]], 7: [K1[3]], 10: [Q0[1]]}
                sched = [
                    (0, 0, [], P00),
                    (1, 0, [], P10),
                    (0, 1, [Q1[1]] + O[0][0:2], None),
                    (1, 1, O[0][2:4] + [Q0[2]], None),
                    (0, 2, [Q1[2]] + O[1][0:2], None),
                    (1, 2, O[1][2:4] + [Q0[3]], None),
                    (0, 3, [Q1[3]] + O[2][0:2], None),
                    (1, 3, O[2][2:4], None),
                ]
                fin = None
                for bi, (pair, qc, fillers, pinned) in enumerate(sched):
                    # last block: PV above even the scores priority so the
                    # PV backlog drains with the exp chain instead of after
                    # it (it gates the tail's norm + final oproj).
                    last = bi == len(sched) - 1
                    fin = attn_block(pair, qc, fillers, pinned=pinned,
                                     carry=fin,
                                     pv_prio=(1 << 21) if last else 0,
                                     split=last)
                fpv, fnorm = fin
                fpv()
                tails[0][0]()
                tails[1][0]()
                fnorm()
                tails[0][1]()
                tails[1][1]()
                tails[2][0]()
                tails[2][1]()
                tails[3][0]()
                tails[3][1]()
    return nc


_CACHE = {}


def _build():
    if "nc" not in _CACHE:
        nc = bacc.Bacc("TRN2", target_bir_lowering=False, debug=False)
        _emit(nc)
        nc.compile()
        _CACHE["nc"] = nc
    return _CACHE["nc"]


def _swiz(a):
    # [C*128, M] -> [128, C*M]: partition p gets row p of every 128-row chunk
    cm, m = a.shape
    c = cm // 128
    return np.ascontiguousarray(
        a.reshape(c, 128, m).transpose(1, 0, 2)).reshape(128, c * m)


def make_in_maps(x, Wq, bq, Wk, bk, Wv, bv, Wo, bo):
    import ml_dtypes
    f32 = np.float32
    bt = ml_dtypes.bfloat16

    def tok_major(xb):
        # [128, (dc, n)] -> [128, (tg, dc, 512)]
        s = _swiz(np.ascontiguousarray(np.asarray(xb, f32).T).astype(bt))
        return np.ascontiguousarray(
            s.reshape(128, DC, TG, 512).transpose(0, 2, 1, 3)
        ).reshape(128, DC * N)

    def pair_major(w):
        # [128, (dc, 256)] -> [128, (pair, dc, 128)]
        return np.ascontiguousarray(
            w.reshape(128, DC, 2, 128).transpose(0, 2, 1, 3)
        ).reshape(128, DC * E)

    xTs = [tok_major(x[b]) for b in range(B)]
    in_maps = []
    for c in range(8):
        b, r0 = c // 4, (c % 4) * E
        rows = slice(r0, r0 + E)
        in_maps.append({
            "xT": xTs[b],
            "wqT": pair_major(_swiz(np.ascontiguousarray(
                np.asarray(Wq, f32)[rows].T).astype(bt))),
            "wkT": pair_major(_swiz(np.ascontiguousarray(
                np.asarray(Wk, f32)[rows].T).astype(bt))),
            "wvT": _swiz(np.ascontiguousarray(np.asarray(Wv, f32)[rows].T).astype(bt)),
            "woT": _swiz(np.ascontiguousarray(np.asarray(Wo, f32)[:, rows].T).astype(bt)),
            "bqk4": np.ascontiguousarray(np.concatenate(
                [np.concatenate([np.asarray(bq, f32)[rows].reshape(2, 128),
                                 np.asarray(bk, f32)[rows].reshape(2, 128)],
                                axis=0),
                 np.eye(4, dtype=f32)], axis=1)),
            "bv1": np.ascontiguousarray(np.asarray(bv, f32)[rows]),
        })
    return in_maps


def kernel(x, Wq, bq, Wk, bk, Wv, bv, Wo, bo, _spmd_kwargs=None):
    nc = _build()
    in_maps = make_in_maps(x, Wq, bq, Wk, bk, Wv, bv, Wo, bo)
    res = run_bass_kernel_spmd(nc, in_maps, core_ids=list(range(8)),
                               **(_spmd_kwargs or {}))
    parts = np.stack([np.asarray(res.results[c]["out"], np.float32)
                      for c in range(8)])
    outv = parts.reshape(B, 4, N, D).sum(axis=1) + np.asarray(bo, np.float32)
    if _spmd_kwargs:
        _CACHE["last_results"] = res
    return outv.astype(np.float32)


# revision 35
# speedup vs baseline: 1.4477x; 1.0378x over previous
"""Multi-head attention (B=2, N=2048, D=1024, H=16) on 8 Trainium2 cores.

Sharding: data-parallel over batch (cores 0-3 -> b=0, cores 4-7 -> b=1) and
tensor-parallel over heads (4 heads per core = 256 of 1024 QKV/O channels).
Each core computes its 4 heads' attention plus a partial output projection;
the host sums the 4 partials per batch and adds bo.

v4 pipeline (per core): the kernel is exp-chain bound (16.8M softmax exps
must stream through ScalarE's 128 lanes at ~1.15us per key-block), so the
win is starting that chain early and never stalling it.
 - x arrives TOKEN-MAJOR: the host lays xT out as (token-group, d-chunk)
   so K/Q projections for tokens 0-511 complete after ~1.5MB of DMA and
   the first scores+exp fire ~15us in, ~14us earlier than a d-chunk-major
   stream.  Remaining x token-groups stream in behind the running
   attention; the K projections for later token-groups and all V/Q/K-pair1
   projections run as PE filler units pinned to specific k-iterations of
   the first two attention blocks, paced to match their DMA arrival.
 - Attention processes a (pair, 512-query-chunk) block at a time.  Per
   k-iter the TWO heads of the pair run their scores matmuls CONCURRENTLY
   in disjoint PE row-groups (K=64 each, tile_position (0,0)/(64,0)) into
   the two halves of one [128,1024] PSUM tile; a single FD=1024 exp on
   ScalarE covers both heads; PV (M=65 with the ones/denominator column)
   runs per head with a one-iter lag.
 - ScalarE's DMA ring is used only during the intro (it is exp-bound
   after); output stores ride sync/gpsimd, plus scalar for the tail.
 - Normalization: den copy, reciprocal_approx_fast, GPSIMD
   partition_broadcast, one tensor_mul reading pv straight from PSUM.
"""

import numpy as np

import concourse.bass as bass
import concourse.bacc as bacc
import concourse.tile as tile
from concourse import mybir
from concourse.bass_utils import run_bass_kernel_spmd

F32 = mybir.dt.float32
BF16 = mybir.dt.bfloat16
AF = mybir.ActivationFunctionType

B, N, D, H, HD = 2, 2048, 1024, 16, 64
E = 256            # channels per core (4 heads * 64)
DC = D // 128      # 8 contraction chunks for projections
NB = N // 128      # 16 token blocks / k chunks
TG = 4             # 512-token groups (x DMA + K/Q projection granularity)
QC = 512           # query chunk
NQC = N // QC      # 4 query chunks
SCALE = 1.0 / np.sqrt(HD)
DT = BF16


def _emit(nc):
    # all big inputs arrive host-swizzled: per-partition contiguous rows so
    # the DMA descriptors are large (partition p owns columns p of every
    # 128-row chunk).  xT is token-group-major: [128, (tg, dc, 512)].
    # wqT/wkT are pair-major: [128, (pair, dc, 128)].
    xT = nc.dram_tensor("xT", [128, DC * N], DT, kind="ExternalInput")
    wqT = nc.dram_tensor("wqT", [128, DC * E], DT, kind="ExternalInput")
    wkT = nc.dram_tensor("wkT", [128, DC * E], DT, kind="ExternalInput")
    wvT = nc.dram_tensor("wvT", [128, DC * E], DT, kind="ExternalInput")
    woT = nc.dram_tensor("woT", [128, 2 * D], DT, kind="ExternalInput")
    # biases ride as FEW-partition rows: a [128, small] DMA costs one
    # descriptor PER PARTITION (~32ns each = ~4us of ring time), which
    # poisons the ring ahead of the x transfers.  [4, 128] is 4 packets.
    bqk4 = nc.dram_tensor("bqk4", [4, 132], F32, kind="ExternalInput")
    # [bv (256) | ones (128)]: the ones row turns the bv spread across
    # partitions into a single K=1 matmul (GPSIMD partition_broadcast would
    # trigger its one-time ucode lib load while the gpsimd DMA ring is
    # still streaming x, stalling ~7us).
    bvo = nc.dram_tensor("bvo", [1, E + 128], F32, kind="ExternalInput")
    out = nc.dram_tensor("out", [N, D], DT, kind="ExternalOutput")

    with tile.TileContext(nc) as tc:
        with tc.tile_pool(name="per", bufs=1) as per, \
             tc.tile_pool(name="wp", bufs=12) as wp, \
             tc.tile_pool(name="dn", bufs=2) as dn, \
             tc.tile_pool(name="up", bufs=2) as up, \
             tc.tile_pool(name="op", bufs=6) as op:

            # ---- persistent SBUF tiles ----
            # x arrives as (token-group, d-chunk-piece) tiles: pieces A/B/C
            # hold d-chunks 0-2 / 3-5 / 6-7 of one 512-token group, each a
            # single contiguous DMA.  Separate tiles per DMA so no false
            # write-order deps serialize them.
            xgA = [per.tile([128, 3, 512], DT, name=f"xgA{t}") for t in range(TG)]
            xgB = [per.tile([128, 3, 512], DT, name=f"xgB{t}") for t in range(TG)]
            xgC = [per.tile([128, 2, 512], DT, name=f"xgC{t}") for t in range(TG)]

            def xap(dc, lo, hi):
                tg, o, oh = lo // 512, lo % 512, hi - (lo // 512) * 512
                assert oh <= 512, "x slice crosses token groups"
                if dc < 3:
                    return xgA[tg][:, dc, o:oh]
                if dc < 6:
                    return xgB[tg][:, dc - 3, o:oh]
                return xgC[tg][:, dc - 6, o:oh]

            wqp = [per.tile([128, DC, 128], DT, name=f"wqp{i}") for i in range(2)]
            wkp = [per.tile([128, DC, 128], DT, name=f"wkp{i}") for i in range(2)]
            wv2 = [per.tile([128, 4, E], DT, name=f"wv2{i}") for i in range(2)]
            wv = [wv2[dc // 4][:, dc % 4, :] for dc in range(DC)]
            wo = per.tile([128, 2, D], DT)            # WoT (e-chunk)
            qt = per.tile([128, 2, N], DT)            # Q^T: (pair, tokens)
            kt = per.tile([128, 2, N], DT)
            vp = per.tile([128, NB, 4, 128], DT)      # V natural + ones col
            at = per.tile([128, 2, N], DT)            # attn^T normalized
            bsrc = per.tile([4, 132], F32)            # bias rows + 4x4 identity
            bqt = per.tile([128, 4], F32)             # [bq p0|p1, bk p0|p1]
            bvr = per.tile([1, E + 128], F32)         # bv row + ones row
            bvb = per.tile([128, E], F32)
            ones64 = per.tile([128, 64], DT)

            qs = [nc.sync, nc.gpsimd]
            # ---- input DMA: need-ordered transfers on the 3 rings ----
            # Deadlines (at ~330GB/s aggregate): wk/wq pair0 + x tg0 by
            # ~13us (attention start), wv by ~15, x tg1 by ~18, x tg2 by
            # ~21, wk pair1 by ~23, x tg3 by ~25, wq pair1 / wo later.
            def xpiece(tg, lo, hi):
                return xT[:, tg * DC * 512 + lo * 512:
                          tg * DC * 512 + hi * 512].rearrange(
                    "p (c n) -> p c n", n=512)

            def wpair(dram, p):
                return dram[:, p * DC * 128:(p + 1) * DC * 128].rearrange(
                    "p (c e) -> p c e", e=128)

            # sync ring
            nc.sync.dma_start(out=wkp[0], in_=wpair(wkT, 0))
            nc.sync.dma_start(out=xgA[0], in_=xpiece(0, 0, 3))
            nc.sync.dma_start(out=xgA[1], in_=xpiece(1, 0, 3))
            nc.sync.dma_start(out=xgA[2], in_=xpiece(2, 0, 3))
            nc.sync.dma_start(out=xgA[3], in_=xpiece(3, 0, 3))
            # scalar ring (free until the first exp ~15us in)
            nc.scalar.dma_start(out=wqp[0], in_=wpair(wqT, 0))
            nc.scalar.dma_start(out=xgB[0], in_=xpiece(0, 3, 6))
            nc.scalar.dma_start(out=xgB[1], in_=xpiece(1, 3, 6))
            nc.scalar.dma_start(out=xgB[2], in_=xpiece(2, 3, 6))
            nc.scalar.dma_start(out=xgB[3], in_=xpiece(3, 3, 6))
            nc.scalar.dma_start(out=wqp[1], in_=wpair(wqT, 1))
            # gpsimd ring
            nc.gpsimd.dma_start(out=bsrc, in_=bqk4[:, :])
            nc.gpsimd.dma_start(out=bvr, in_=bvo[:, :])
            nc.gpsimd.dma_start(out=xgC[0], in_=xpiece(0, 6, 8))
            nc.gpsimd.dma_start(out=wv2[0], in_=wvT[:, 0:4 * E].rearrange(
                "p (c e) -> p c e", e=E))
            nc.gpsimd.dma_start(out=wv2[1], in_=wvT[:, 4 * E:8 * E].rearrange(
                "p (c e) -> p c e", e=E))
            nc.gpsimd.dma_start(out=xgC[1], in_=xpiece(1, 6, 8))
            nc.gpsimd.dma_start(out=xgC[2], in_=xpiece(2, 6, 8))
            nc.gpsimd.dma_start(out=wkp[1], in_=wpair(wkT, 1))
            nc.gpsimd.dma_start(out=xgC[3], in_=xpiece(3, 6, 8))
            nc.gpsimd.dma_start(out=wo, in_=woT.rearrange(
                "p (c e) -> p c e", e=D))

            # ---- on-chip constants: ones column ----
            nc.gpsimd.memset(ones64, 1.0)
            nc.vector.tensor_copy(vp[:, :, :, HD:HD + 1].rearrange(
                "p a b o -> p (a b o)"), ones64)

            # ---- pre-phase: bias transpose + K pair0 tg0 + Q pair0 g0 ----
            # (all other projections run as pinned/paced fillers inside the
            # attention blocks, matched to DMA arrival order)
            with tc.tile_pool(name="pre", bufs=1, space="PSUM") as pre:
                pb = pre.tile([128, 4], F32, tag="p2", name="pb")
                nc.tensor.matmul(pb, bsrc[:, 0:128], bsrc[:, 128:132],
                                 start=True, stop=True)
                nc.vector.tensor_copy(bqt, pb)
                bqs = bqt[:, 0:2]
                bks = bqt[:, 2:4]
                pbv = pre.tile([128, E], F32, tag="p3", name="pbv")
                nc.tensor.matmul(pbv, bvr[0:1, E:E + 128], bvr[0:1, 0:E],
                                 start=True, stop=True)
                nc.vector.tensor_copy(bvb, pbv)
                pk0 = pre.tile([128, 512], F32, tag="p0", name="pk0")
                pq0 = pre.tile([128, 512], F32, tag="p1", name="pq0")
                for dc in range(DC):
                    nc.tensor.matmul(
                        pk0, wkp[0][:, dc, :], xap(dc, 0, 512),
                        start=(dc == 0), stop=(dc == DC - 1))
                for dc in range(DC):
                    nc.tensor.matmul(
                        pq0, wqp[0][:, dc, :], xap(dc, 0, 512),
                        start=(dc == 0), stop=(dc == DC - 1))
                with tc.high_priority(offset=1 << 19):
                    nc.vector.tensor_scalar_add(kt[:, 0, 0:512], pk0,
                                                bks[:, 0:1])
                    nc.vector.tensor_scalar_add(qt[:, 0, 0:512], pq0,
                                                bqs[:, 0:1])

            with tc.tile_pool(name="ps", bufs=1, space="PSUM") as ps:
                pj_n = [0]

                def pj_tag():
                    pj_n[0] += 1
                    return ("pjA", "pjB")[pj_n[0] % 2]

                # ---- filler units (1-bank psum groups on pj tags) ----
                # K/Q projection groups run at boosted priority: they feed
                # upcoming scores and must not queue behind the
                # V-projection / PV backlog.
                def proj_group(wpt, dst, bias, pair, n4):
                    def emit():
                        with tc.high_priority(offset=1 << 19):
                            pt = ps.tile([128, 512], F32, tag=pj_tag(),
                                         name="ppj")
                            for dc in range(DC):
                                nc.tensor.matmul(
                                    pt[:, :],
                                    wpt[:, dc, :],
                                    xap(dc, n4 * 512, (n4 + 1) * 512),
                                    start=(dc == 0), stop=(dc == DC - 1))
                            nc.vector.tensor_scalar_add(
                                dst[:, pair, n4 * 512:(n4 + 1) * 512], pt[:, :],
                                bias[:, pair:pair + 1])
                    return emit

                def vnat_group(nb):
                    def emit():
                        pt = ps.tile([128, E], F32, tag=pj_tag(), name="pvn")
                        for dc in range(DC):
                            nc.tensor.matmul(
                                pt[:, :],
                                xap(dc, nb * 128, (nb + 1) * 128),
                                wv[dc],
                                start=(dc == 0), stop=(dc == DC - 1))
                        nc.vector.tensor_add(
                            vp[:, nb, :, 0:HD],
                            pt.rearrange("p (h d) -> p h d", h=4),
                            bvb.rearrange("p (h d) -> p h d", h=4))
                    return emit

                o_n = [0]

                def oproj_unit(nb, evict="dve", tags=None, ring=None):
                    # both D-halves of a 128-token block -> one contiguous
                    # [128, 1024] row store
                    def emit():
                        ot = op.tile([128, 1024], DT, tag="ot", name="ot")
                        pos = [ps.tile([128, 512], F32,
                                       tag=(tags[h] if tags else pj_tag()),
                                       name="po")
                               for h in range(2)]
                        # ec-outer: each at-chunk stationary serves both
                        # D-halves consecutively
                        for ec in range(2):
                            for half in range(2):
                                nc.tensor.matmul(
                                    pos[half][:, :],
                                    at[:, ec, nb * 128:(nb + 1) * 128],
                                    wo[:, ec, half * 512:(half + 1) * 512],
                                    start=(ec == 0), stop=(ec == 1))
                        for half in range(2):
                            if evict == "dve":
                                nc.vector.tensor_copy(
                                    ot[:, half * 512:(half + 1) * 512],
                                    pos[half])
                            else:
                                nc.scalar.copy(
                                    ot[:, half * 512:(half + 1) * 512],
                                    pos[half])
                        o_n[0] += 1
                        (ring or qs[o_n[0] % 2]).dma_start(
                            out=out[nb * 128:(nb + 1) * 128, :], in_=ot)
                    return emit

                # ---- one (pair, qc) attention block: 16 k-iters ----
                # Returns a `finish` closure (last PV pair + normalization);
                # the caller runs it after the NEXT block's first k-iter so
                # ScalarE never stalls across block boundaries.  `pinned`
                # maps k-iter -> filler units that must run at that iter
                # (used to match DMA arrival order in the first blocks).
                def attn_block(pair, qc, fillers, pinned=None, carry=None,
                               pv_prio=0, split=False):
                    q0 = qc * QC
                    fi = 0
                    pvs = [ps.tile([HD + 1, QC], F32, tag=t, name=t)
                           for t in ("pvA", "pvB")]
                    wtiles = {}
                    for k in range(NB):
                        st = ps.tile([128, 1024], F32,
                                     tag=("s0", "s1")[k % 2], name="st")
                        # scores for both heads back-to-back at max priority
                        # so they sit adjacent in the PE queue and overlap in
                        # disjoint row-groups of the array.
                        with tc.high_priority(offset=1 << 20):
                            for hh in range(2):
                                p0 = hh * HD
                                nc.tensor.matmul(
                                    st[:, hh * QC:(hh + 1) * QC],
                                    kt[p0:p0 + HD, pair, k * 128:(k + 1) * 128],
                                    qt[p0:p0 + HD, pair, q0:q0 + QC],
                                    start=True, stop=True,
                                    tile_position=(p0, 0))
                        w = wp.tile([128, 1024], DT, tag="w", name="w")
                        nc.scalar.activation(w, st, AF.Exp, scale=SCALE)
                        wtiles[k] = w
                        if k == 0 and carry is not None:
                            carry()
                            carry = None
                        if pinned:
                            for u in pinned.get(k, []):
                                u()
                        while fi < (k + 1) * len(fillers) // NB:
                            fillers[fi]()
                            fi += 1
                        if k > 0:
                            wprev = wtiles.pop(k - 1)
                            with tc.high_priority(offset=pv_prio):
                                for hh in range(2):
                                    nc.tensor.matmul(
                                        pvs[hh][:, :],
                                        vp[:, k - 1, 2 * pair + hh, 0:HD + 1],
                                        wprev[:, hh * QC:(hh + 1) * QC],
                                        start=(k - 1 == 0), stop=False)
                    wlast = wtiles.pop(NB - 1)

                    def finish_pv():
                        for hh in range(2):
                            nc.tensor.matmul(
                                pvs[hh][:, :],
                                vp[:, NB - 1, 2 * pair + hh, 0:HD + 1],
                                wlast[:, hh * QC:(hh + 1) * QC],
                                start=False, stop=True)

                    def finish_norm():
                        # normalize both heads, ops interleaved so the GP
                        # broadcasts overlap the DVE chain; the at-mul reads
                        # pv straight from PSUM.
                        den = [dn.tile([1, QC], F32, tag=f"den{h}",
                                       name=f"den{h}") for h in range(2)]
                        rec = [dn.tile([1, QC], F32, tag=f"rec{h}",
                                       name=f"rec{h}") for h in range(2)]
                        bcr = [up.tile([HD, QC], F32, tag=f"bcr{h}",
                                       name=f"bcr{h}") for h in range(2)]
                        for hh in range(2):
                            nc.vector.tensor_copy(den[hh],
                                                  pvs[hh][HD:HD + 1, :])
                        for hh in range(2):
                            nc.vector.reciprocal_approx_fast(rec[hh],
                                                             den[hh])
                            nc.gpsimd.partition_broadcast(bcr[hh],
                                                          rec[hh][0:1, :])
                        for hh in range(2):
                            p0 = hh * HD
                            nc.vector.tensor_mul(
                                at[p0:p0 + HD, pair, q0:q0 + QC],
                                pvs[hh][0:HD, :], bcr[hh])

                    def finish():
                        finish_pv()
                        finish_norm()
                    return (finish_pv, finish_norm) if split else finish

                # ---- filler inventory ----
                V = [vnat_group(i) for i in range(NB)]
                K0 = [proj_group(wkp[0], kt, bks, 0, g) for g in range(4)]
                K1 = [proj_group(wkp[1], kt, bks, 1, g) for g in range(4)]
                Q0 = [proj_group(wqp[0], qt, bqs, 0, g) for g in range(4)]
                Q1 = [proj_group(wqp[1], qt, bqs, 1, g) for g in range(4)]
                O = [[oproj_unit(4 * qc + nb) for nb in range(4)]
                     for qc in range(3)]

                # tail units (qc=3, after the last block): pair-0 oproj
                # halves can run DURING the final normalization (they only
                # need `at` pair 0, written two blocks earlier) — keeping
                # the PE warm so HAM doesn't re-throttle; pair-1 halves,
                # evictions (DVE + ScalarE in parallel) and per-half
                # stores follow the norm.
                def tail_unit(nb, tags, rings):
                    state = {}

                    def begin():
                        pos = [ps.tile([128, 512], F32, tag=tags[h],
                                       name="po") for h in range(2)]
                        state["pos"] = pos
                        for half in range(2):
                            nc.tensor.matmul(
                                pos[half][:, :],
                                at[:, 0, nb * 128:(nb + 1) * 128],
                                wo[:, 0, half * 512:(half + 1) * 512],
                                start=True, stop=False)

                    def end():
                        pos = state["pos"]
                        ot = op.tile([128, 1024], DT, tag="ot", name="ot")
                        for half in range(2):
                            nc.tensor.matmul(
                                pos[half][:, :],
                                at[:, 1, nb * 128:(nb + 1) * 128],
                                wo[:, 1, half * 512:(half + 1) * 512],
                                start=False, stop=True)
                        for half in range(2):
                            if half == 0:
                                nc.vector.tensor_copy(
                                    ot[:, 0:512], pos[0])
                            else:
                                nc.scalar.copy(
                                    ot[:, 512:1024], pos[1])
                            rings[half].dma_start(
                                out=out[nb * 128:(nb + 1) * 128,
                                        half * 512:(half + 1) * 512],
                                in_=ot[:, half * 512:(half + 1) * 512])
                    return begin, end

                tails = [tail_unit(12, ("pjA", "pjB"), (nc.sync, nc.gpsimd)),
                         tail_unit(13, ("s0", "s1"), (nc.scalar, nc.sync)),
                         tail_unit(14, ("pvA", "pvB"), (nc.gpsimd, nc.scalar)),
                         tail_unit(15, ("pjA", "pjB"), (nc.sync, nc.gpsimd))]

                # Block (0,0): V[j] pinned at iter j+1 feeds PV(j) just in
                # time; K0[tg] pinned ahead of scores iter 4*tg, paced to
                # x token-group arrival.  Block (1,0): K1[g] ahead of its
                # scores iter 4*g.
                P00 = {1: [V[0]], 2: [V[1], K0[1]], 3: [V[2]], 4: [V[3]],
                       5: [V[4]], 6: [V[5], K0[2]], 7: [V[6]], 8: [V[7]],
                       9: [V[8], K0[3]], 10: [V[9]], 11: [V[10]],
                       12: [V[11], K1[0]], 13: [V[12]], 14: [V[13], Q1[0]],
                       15: [V[14], V[15]]}
                P10 = {0: [K1[1]], 3: [K1[2]], 7: [K1[3]], 10: [Q0[1]]}
                sched = [
                    (0, 0, [], P00),
                    (1, 0, [], P10),
                    (0, 1, [Q1[1]] + O[0][0:2], None),
                    (1, 1, O[0][2:4] + [Q0[2]], None),
                    (0, 2, [Q1[2]] + O[1][0:2], None),
                    (1, 2, O[1][2:4] + [Q0[3]], None),
                    (0, 3, [Q1[3]] + O[2][0:2], None),
                    (1, 3, O[2][2:4], None),
                ]
                fin = None
                for bi, (pair, qc, fillers, pinned) in enumerate(sched):
                    # last block: PV above even the scores priority so the
                    # PV backlog drains with the exp chain instead of after
                    # it (it gates the tail's norm + final oproj).
                    last = bi == len(sched) - 1
                    fin = attn_block(pair, qc, fillers, pinned=pinned,
                                     carry=fin,
                                     pv_prio=(1 << 21) if last else 0,
                                     split=last)
                fpv, fnorm = fin
                fpv()
                tails[0][0]()
                tails[1][0]()
                fnorm()
                tails[0][1]()
                tails[1][1]()
                tails[2][0]()
                tails[2][1]()
                tails[3][0]()
                tails[3][1]()
    return nc


_CACHE = {}


def _build():
    if "nc" not in _CACHE:
        nc = bacc.Bacc("TRN2", target_bir_lowering=False, debug=False)
        _emit(nc)
        nc.compile()
        _CACHE["nc"] = nc
    return _CACHE["nc"]


def _swiz(a):
    # [C*128, M] -> [128, C*M]: partition p gets row p of every 128-row chunk
    cm, m = a.shape
    c = cm // 128
    return np.ascontiguousarray(
        a.reshape(c, 128, m).transpose(1, 0, 2)).reshape(128, c * m)


def make_in_maps(x, Wq, bq, Wk, bk, Wv, bv, Wo, bo):
    import ml_dtypes
    f32 = np.float32
    bt = ml_dtypes.bfloat16

    def tok_major(xb):
        # [128, (dc, n)] -> [128, (tg, dc, 512)]
        s = _swiz(np.ascontiguousarray(np.asarray(xb, f32).T).astype(bt))
        return np.ascontiguousarray(
            s.reshape(128, DC, TG, 512).transpose(0, 2, 1, 3)
        ).reshape(128, DC * N)

    def pair_major(w):
        # [128, (dc, 256)] -> [128, (pair, dc, 128)]
        return np.ascontiguousarray(
            w.reshape(128, DC, 2, 128).transpose(0, 2, 1, 3)
        ).reshape(128, DC * E)

    xTs = [tok_major(x[b]) for b in range(B)]
    in_maps = []
    for c in range(8):
        b, r0 = c // 4, (c % 4) * E
        rows = slice(r0, r0 + E)
        in_maps.append({
            "xT": xTs[b],
            "wqT": pair_major(_swiz(np.ascontiguousarray(
                np.asarray(Wq, f32)[rows].T).astype(bt))),
            "wkT": pair_major(_swiz(np.ascontiguousarray(
                np.asarray(Wk, f32)[rows].T).astype(bt))),
            "wvT": _swiz(np.ascontiguousarray(np.asarray(Wv, f32)[rows].T).astype(bt)),
            "woT": _swiz(np.ascontiguousarray(np.asarray(Wo, f32)[:, rows].T).astype(bt)),
            "bqk4": np.ascontiguousarray(np.concatenate(
                [np.concatenate([np.asarray(bq, f32)[rows].reshape(2, 128),
                                 np.asarray(bk, f32)[rows].reshape(2, 128)],
                                axis=0),
                 np.eye(4, dtype=f32)], axis=1)),
            "bvo": np.ascontiguousarray(np.concatenate(
                [np.asarray(bv, f32)[rows],
                 np.ones(128, f32)]).reshape(1, E + 128)),
        })
    return in_maps


def kernel(x, Wq, bq, Wk, bk, Wv, bv, Wo, bo, _spmd_kwargs=None):
    nc = _build()
    in_maps = make_in_maps(x, Wq, bq, Wk, bk, Wv, bv, Wo, bo)
    res = run_bass_kernel_spmd(nc, in_maps, core_ids=list(range(8)),
                               **(_spmd_kwargs or {}))
    parts = np.stack([np.asarray(res.results[c]["out"], np.float32)
                      for c in range(8)])
    outv = parts.reshape(B, 4, N, D).sum(axis=1) + np.asarray(bo, np.float32)
    if _spmd_kwargs:
        _CACHE["last_results"] = res
    return outv.astype(np.float32)
